# revision 11
# baseline (speedup 1.0000x reference)
"""AxialSpaceTimeTransformer on 8 TRN2 NeuronCores — single full Bass kernel.

Sharding (8-way, single chip):
  * t-domain: core c holds frames t in [4c, 4c+4) for both batches.
    Space-attention (over s) and FF are core-local here.
  * s-domain: core c holds spatial positions s in [32c, 32c+32).
    Causal time-attention (over t) is core-local here.

The ENTIRE network (rv projection, 6 space layers, 2 causal time layers
with rotary, all FFs, final rmsnorm) runs as ONE Bass kernel invoked once
per call; the four t<->s reshardings are in-kernel AllToAll collectives
through DRAM bounce buffers.  No XLA compute stages remain.
"""

import os
import sys
import types

import numpy as np

if "/opt/trn_rl_repo" not in sys.path:
    sys.path.insert(0, "/opt/trn_rl_repo")

# -- antenv.axon_hooks shim (agent image lacks it; bass_utils wants it) --
import antenv  # noqa: E402

if not hasattr(antenv, "axon_hooks"):
    _hooks = types.ModuleType("antenv.axon_hooks")
    _hooks._hook = None
    _hooks.set_axon_ntff_profile_hook = lambda h: setattr(_hooks, "_hook", h)
    _hooks.get_axon_ntff_profile_hook = lambda: _hooks._hook
    sys.modules["antenv.axon_hooks"] = _hooks
    antenv.axon_hooks = _hooks
    try:
        from trn_agent_boot.trn_boot import _ntff_profile_via_ctypes

        _hooks.set_axon_ntff_profile_hook(
            _ntff_profile_via_ctypes("/opt/axon/libaxon_pjrt.so")
        )
    except Exception:
        pass

import jax  # noqa: E402
import jax.numpy as jnp  # noqa: E402
from jax.sharding import Mesh, NamedSharding, PartitionSpec as P  # noqa: E402
from jax.experimental.shard_map import shard_map  # noqa: E402

DIM = 768
DEPTH = 8
HEADS = 12
DH = 64
DFF = 2048
SOFTCLAMP = 50.0
B, T, S = 2, 32, 256
EPS = 1e-6
NC = 8
TL = T // NC  # 4 frames/core (t-domain)
SL = S // NC  # 32 positions/core (s-domain)
NTOK = B * TL * S  # 2048 tokens per core in either domain


def _round_f32r(x):
    """fp32 -> fp32r (13 explicit mantissa bits, RNE) rounding on host."""
    u = np.ascontiguousarray(x, dtype=np.float32).view(np.uint32)
    lsb = (u >> 10) & 1
    r = (u + 0x1FF + lsb) & np.uint32(0xFFFFFC00)
    return r.view(np.float32).copy()


def _make_rotary(n):
    inv = 1.0 / (10000.0 ** (np.arange(0, DH, 2, dtype=np.float32) / DH))
    f = np.arange(n, dtype=np.float32)[:, None] * inv[None, :]
    return np.concatenate([f, f], axis=-1)  # (n, DH)


def _pack_inputs(inputs):
    """Host-side weight folding/packing for the bass kernel (np arrays)."""
    f32 = np.float32
    SP = [0, 1, 2, 4, 5, 6]
    TM = [3, 7]
    anw = np.asarray(inputs["attn_norm_w"], f32)[:, :, None]
    fnw = np.asarray(inputs["ff_norm_w"], f32)[:, :, None]
    Wq = np.asarray(inputs["Wq"], f32) * anw
    Wk = np.asarray(inputs["Wk"], f32) * anw
    Wv = np.asarray(inputs["Wv"], f32) * anw
    Wo = np.asarray(inputs["Wo"], f32)
    Wmg = np.concatenate(
        [
            np.asarray(inputs["Wmix"], f32) * anw,
            np.asarray(inputs["Wg"], f32) * anw,
        ],
        axis=2,
    )  # (8, 768, 24)
    # k scale applied after l2norm; folds sqrt(DH), 1/sqrt(DH) and 1/softclamp
    kg = ((np.asarray(inputs["k_gamma"], f32) + 1.0) / SOFTCLAMP).reshape(8, 768)
    Win = np.asarray(inputs["Win"], f32) * fnw
    Wout = np.asarray(inputs["Wout"], f32)

    g = {
        "Wq6": _round_f32r(Wq[SP]),
        "Wk6": _round_f32r(Wk[SP]),
        "Wv6": _round_f32r(Wv[SP]),
        "Wo6": _round_f32r(Wo[SP]),
        "Wmg6": _round_f32r(Wmg[SP]),
        "kg6": kg[SP].astype(f32),
        "Win6": _round_f32r(Win[SP]),
        "Wout6": _round_f32r(Wout[SP]),
        "WqT": _round_f32r(Wq[TM]),
        "WkT": _round_f32r(Wk[TM]),
        "WvT": _round_f32r(Wv[TM]),
        "WoT": _round_f32r(Wo[TM]),
        "WmgT": _round_f32r(Wmg[TM]),
        "kgT": kg[TM].astype(f32),
        "WinT": _round_f32r(Win[TM]),
        "WoutT": _round_f32r(Wout[TM]),
        "vrW": _round_f32r(
            np.asarray(inputs["vr_norm_w"], f32)[:, None]
            * np.asarray(inputs["vr_W"], f32)
        ),
    }
    # rotary tables, feature-major: [p in 0..128 = 2 heads x 64 d, n in 0..256]
    rot = _make_rotary(T)  # (32, 64)
    pp = np.arange(128)[:, None] % 64
    nn = np.arange(256)[None, :] % T
    g["rotc"] = np.cos(rot[nn, pp]).astype(f32)
    g["rots"] = np.sin(rot[nn, pp]).astype(f32)
    # rotate-half permutation as a matmul stationary: Pq = pmat.T @ q_f
    pm = np.zeros((128, 128), f32)
    for i in range(128):
        base, d = (i // 64) * 64, i % 64
        if d < 32:
            pm[base + d + 32, i] = -1.0
        else:
            pm[base + d - 32, i] = 1.0
    g["pmat"] = pm
    # block-diag causal mask [k-part, q-free] over 4 seqs of 32
    kp = np.arange(128)[:, None]
    qc = np.arange(128)[None, :]
    g["maskf"] = (
        ((kp // 32 == qc // 32) & (kp % 32 <= qc % 32)).astype(f32)
    )
    return g


# ---------------------------------------------------------------------------
# cached compiled pipeline
# ---------------------------------------------------------------------------
_PIPE = None


def _build_pipeline(inputs):
    devs = jax.devices()[:NC]
    mesh = Mesh(np.asarray(devs), ("core",))
    shard = NamedSharding(mesh, P("core"))

    nc, in_names, out_names, out_avals = build_full()
    from concourse import bass2jax
    from concourse.bass2jax import _bass_exec_p

    bind_names = tuple(in_names + out_names)
    pid_name = nc.partition_id_tensor.name if nc.partition_id_tensor else None
    full_names = bind_names + ((pid_name,) if pid_name else ())

    def bass_body(*args):
        ops = list(args)
        if pid_name is not None:
            ops.append(bass2jax.partition_id_tensor())
        outs = _bass_exec_p.bind(
            *ops,
            out_avals=tuple(out_avals),
            in_names=full_names,
            out_names=tuple(out_names),
            lowering_input_output_aliases=(),
            sim_require_finite=True,
            sim_require_nnan=True,
            nc=nc,
        )
        return tuple(outs)

    percore = {"x_in", "x_out"}
    in_specs = tuple(P("core") if n in percore else P() for n in bind_names)
    out_specs = (P("core"),) * len(out_names)
    nout = len(out_names)
    bass_jit = jax.jit(
        shard_map(bass_body, mesh=mesh, in_specs=in_specs,
                  out_specs=out_specs, check_rep=False),
        donate_argnums=tuple(range(len(bind_names) - nout, len(bind_names))),
    )

    packs = {k: jnp.asarray(v) for k, v in _pack_inputs(inputs).items()}

    zjit = jax.jit(
        lambda: jnp.zeros((NC * NTOK, DIM), jnp.float32),
        out_shardings=shard,
    )

    def run(tok_flat):
        tok = jax.device_put(tok_flat, shard)
        ops = []
        for nme in in_names:
            if nme == "x_in":
                ops.append(tok)
            else:
                ops.append(packs[nme])
        (out,) = bass_jit(*ops, zjit())
        return out

    run.stages = {}
    return run


def shard_tokens(tokens):
    """(B,T,S,D) -> (NC*2048, D) t-domain rows: tile=(th,tl,b), p=(jh,sll,slh)."""
    A = tokens.reshape(B, NC, TL, 2, 4, 8, 4, DIM)  # b c tl th jh slh sll d
    A = A.transpose(1, 3, 2, 0, 4, 6, 5, 7)  # c th tl b jh sll slh d
    return np.ascontiguousarray(A).reshape(NC * NTOK, DIM)


def unshard_out(out):
    """(NC*2048, D) s-domain rows: tile=(slh,b), p=(sll,c,tl) -> (B,T,S,D)."""
    rec = out.reshape(NC, 8, 2, 4, 8, 4, DIM)  # core slh b sll c tl d
    rec = rec.transpose(2, 4, 5, 0, 1, 3, 6)  # b c tl core slh sll d
    return np.ascontiguousarray(rec).reshape(B, T, S, DIM)


def kernel(**inputs):
    global _PIPE
    tokens = np.asarray(inputs["tokens"], dtype=np.float32)
    tok_bt = shard_tokens(tokens)

    if _PIPE is None:
        _PIPE = _build_pipeline(inputs)
    out = np.asarray(jax.block_until_ready(_PIPE(jnp.asarray(tok_bt))))

    out = unshard_out(out)
    out = out * np.asarray(inputs["final_norm_w"], np.float32)
    return np.ascontiguousarray(out.astype(np.float32))


# ---------------------------------------------------------------------------
# Bass kernel
# ---------------------------------------------------------------------------
from contextlib import ExitStack  # noqa: E402

import concourse.bacc as bacc  # noqa: E402
import concourse.mybir as mybir  # noqa: E402
import concourse.tile as tile  # noqa: E402
from concourse.bass import ds  # noqa: E402
from concourse.masks import make_identity  # noqa: E402

F32 = mybir.dt.float32
F32R = mybir.dt.float32r
BF16 = mybir.dt.bfloat16
I32 = mybir.dt.int32
AF = mybir.ActivationFunctionType
OP = mybir.AluOpType

NT = 16  # token tiles (2048 tokens)
KT = 6  # 768 / 128 feature tiles
H = 12
RG = [list(range(NC))]


def _emit_rsqrt(nc, pool, out, in_, scale, bias, guard):
    """out = 1/sqrt(max(in_*scale + bias, guard)); quake seed + 3 Newton."""
    shp = [128, in_.shape[1]]
    m = pool.tile(shp, F32, name="rs_m", tag="rs_m")
    nc.vector.tensor_scalar(m[:], in_, scale, bias, op0=OP.mult, op1=OP.add)
    nc.vector.tensor_scalar_max(m[:], m[:], guard)
    yi = pool.tile(shp, I32, name="rs_yi", tag="rs_yi")
    nc.vector.tensor_scalar(
        yi[:], m[:].bitcast(I32), 1, None, op0=OP.arith_shift_right
    )
    nc.vector.tensor_scalar(
        yi[:], yi[:], -1, 0x5F3759DF, op0=OP.mult, op1=OP.add
    )
    y = yi[:].bitcast(F32)
    half = pool.tile(shp, F32, name="rs_half", tag="rs_half")
    nc.vector.tensor_scalar_mul(half[:], m[:], 0.5)
    t1 = pool.tile(shp, F32, name="rs_t1", tag="rs_t1")
    for it in range(3):
        nc.vector.tensor_tensor(t1[:], y, y, op=OP.mult)
        nc.vector.tensor_tensor(t1[:], t1[:], half[:], op=OP.mult)
        nc.vector.tensor_scalar(t1[:], t1[:], -1.0, 1.5, op0=OP.mult, op1=OP.add)
        if it < 2:
            nc.vector.tensor_tensor(y, y, t1[:], op=OP.mult)
        else:
            nc.vector.tensor_tensor(out, y, t1[:], op=OP.mult)
    return out


def build_full():
    nc = bacc.Bacc(None, target_bir_lowering=False, num_devices=NC)

    x_in = nc.dram_tensor("x_in", [NTOK, DIM], F32, kind="ExternalInput")
    Wq6 = nc.dram_tensor("Wq6", [6, 768, 768], F32R, kind="ExternalInput")
    Wk6 = nc.dram_tensor("Wk6", [6, 768, 768], F32R, kind="ExternalInput")
    Wv6 = nc.dram_tensor("Wv6", [6, 768, 768], F32R, kind="ExternalInput")
    Wo6 = nc.dram_tensor("Wo6", [6, 768, 768], F32R, kind="ExternalInput")
    Wmg6 = nc.dram_tensor("Wmg6", [6, 768, 24], F32R, kind="ExternalInput")
    kg6 = nc.dram_tensor("kg6", [6, 768], F32, kind="ExternalInput")
    Win6 = nc.dram_tensor("Win6", [6, 768, 4096], F32R, kind="ExternalInput")
    Wout6 = nc.dram_tensor("Wout6", [6, 2048, 768], F32R, kind="ExternalInput")
    WqT = nc.dram_tensor("WqT", [2, 768, 768], F32R, kind="ExternalInput")
    WkT = nc.dram_tensor("WkT", [2, 768, 768], F32R, kind="ExternalInput")
    WvT = nc.dram_tensor("WvT", [2, 768, 768], F32R, kind="ExternalInput")
    WoT = nc.dram_tensor("WoT", [2, 768, 768], F32R, kind="ExternalInput")
    WmgT = nc.dram_tensor("WmgT", [2, 768, 24], F32R, kind="ExternalInput")
    kgT = nc.dram_tensor("kgT", [2, 768], F32, kind="ExternalInput")
    WinT = nc.dram_tensor("WinT", [2, 768, 4096], F32R, kind="ExternalInput")
    WoutT = nc.dram_tensor("WoutT", [2, 2048, 768], F32R, kind="ExternalInput")
    vrW = nc.dram_tensor("vrW", [768, 768], F32R, kind="ExternalInput")
    rotc_i = nc.dram_tensor("rotc", [128, 256], F32, kind="ExternalInput")
    rots_i = nc.dram_tensor("rots", [128, 256], F32, kind="ExternalInput")
    pmat_i = nc.dram_tensor("pmat", [128, 128], F32, kind="ExternalInput")
    mask_i = nc.dram_tensor("maskf", [128, 128], F32, kind="ExternalInput")
    x_out = nc.dram_tensor("x_out", [NTOK, DIM], F32, kind="ExternalOutput")

    with tile.TileContext(nc) as tc:
        with ExitStack() as top:
            const = top.enter_context(tc.tile_pool(name="const", bufs=1))
            dram = top.enter_context(
                tc.tile_pool(name="dramp", bufs=1, space="DRAM")
            )
            xpool = top.enter_context(tc.tile_pool(name="xpool", bufs=1))

            x_sb = xpool.tile([128, NT, 768], F32, name="x_sb")
            nc.sync.dma_start(
                x_sb[:], x_in[:].rearrange("(t p) d -> p t d", p=128)
            )

            ident_f = const.tile([128, 128], F32, name="ident_f")
            make_identity(nc, ident_f)
            ident = const.tile([128, 128], F32R, name="ident")
            nc.vector.tensor_copy(ident[:], ident_f[:])

            ld_f = const.tile([128, 128], F32, name="ld_f")
            nc.sync.dma_start(ld_f[:], pmat_i[:])
            pmat = const.tile([128, 128], F32R, name="pmat_t")
            nc.vector.tensor_copy(pmat[:], ld_f[:])
            mk_f = const.tile([128, 128], F32, name="mk_f")
            nc.sync.dma_start(mk_f[:], mask_i[:])
            mask_b = const.tile([128, 128], BF16, name="mask_b")
            nc.vector.tensor_copy(mask_b[:], mk_f[:])
            rotc = const.tile([128, 256], F32, name="rotc_t")
            nc.sync.dma_start(rotc[:], rotc_i[:])
            rots = const.tile([128, 256], F32, name="rots_t")
            nc.sync.dma_start(rots[:], rots_i[:])

            # DRAM bounce buffers (chunk layout sll,slh,tl,b,d)
            rv_t_d = dram.tile([NTOK, 768], F32, name="rv_t_d")
            rv_ain = dram.tile([NC, 4, 8, 4, 2, 768], F32, name="rv_ain")
            rv_s_d = dram.tile([NC, 4, 8, 4, 2, 768], F32, name="rv_s_d")
            xa_in = dram.tile([NC, 4, 8, 4, 2, 768], F32, name="xa_in")
            xa_out = dram.tile([NC, 4, 8, 4, 2, 768], F32, name="xa_out")

            # rv in s-domain: per-group view, dims (slh | sll c tl | b | d)
            rv_s_view = rv_s_d[:].rearrange(
                "c sll slh tl b d -> slh sll c tl b d"
            )

            # ---- pre: rv projection + rv all-to-all ------------------------
            _pre_rv(nc, tc, x_sb, ident, vrW, rv_t_d, rv_ain, rv_s_d)

            # ---- layers ----------------------------------------------------
            for li in range(3):
                _attn_layer(nc, tc, li, x_sb, ident, rv_t_d, Wq6, Wk6, Wv6,
                            Wo6, Wmg6, kg6)
                _ff_layer(nc, tc, li, x_sb, ident, Win6, Wout6, "s")

            _a2a_t2s(nc, tc, x_sb, xa_in, xa_out)

            _time_layer(nc, tc, 0, x_sb, ident, pmat, rotc, rots, mask_b,
                        rv_s_view, WqT, WkT, WvT, WoT, WmgT, kgT)
            _ff_layer(nc, tc, 0, x_sb, ident, WinT, WoutT, "t")

            _a2a_s2t(nc, tc, x_sb, xa_in, xa_out)

            for li in range(3, 6):
                _attn_layer(nc, tc, li, x_sb, ident, rv_t_d, Wq6, Wk6, Wv6,
                            Wo6, Wmg6, kg6)
                _ff_layer(nc, tc, li, x_sb, ident, Win6, Wout6, "s")

            _a2a_t2s(nc, tc, x_sb, xa_in, xa_out)

            _time_layer(nc, tc, 1, x_sb, ident, pmat, rotc, rots, mask_b,
                        rv_s_view, WqT, WkT, WvT, WoT, WmgT, kgT)
            _ff_layer(nc, tc, 1, x_sb, ident, WinT, WoutT, "t")

            _final_norm(nc, tc, x_sb, x_out)

    nc.compile()

    in_names = []
    out_names = []
    out_avals = []

    pname = nc.partition_id_tensor.name if nc.partition_id_tensor else None
    for alloc in nc.m.functions[0].allocations:
        if not isinstance(alloc, mybir.MemoryLocationSet):
            continue
        if not alloc.memorylocations:
            continue
        name = alloc.memorylocations[0].name
        if alloc.kind == "ExternalInput" and name != pname:
            in_names.append(name)
        elif alloc.kind == "ExternalOutput":
            out_names.append(name)
            out_avals.append(
                jax.core.ShapedArray(
                    tuple(alloc.tensor_shape), mybir.dt.np(alloc.dtype)
                )
            )
    return nc, in_names, out_names, out_avals


# ---------------------------------------------------------------------------
# all-to-all helpers.  Bounce buffers are [NC, 4(sll), 8(slh), 4(tl), 2(b), d]
# (chunk layout sll,slh,tl,b,d).  t-domain sbuf: tile=(th,tl,b), p=(jh,sll,slh)
# with s = 128*th + 32*jh + 4*slh + sll.  s-domain sbuf: tile=(slh,b),
# p=(sll,c,tl) with t = 4*c + tl, sl = 4*slh + sll.
# ---------------------------------------------------------------------------
def _send_t2s(nc, src_sb, xa_in):
    """t-domain SBUF -> bounce chunks (8 DMAs)."""
    xv = src_sb[:].rearrange("p (th tl b) d -> p th tl b d", th=2, tl=4)
    for th in range(2):
        for jh in range(4):
            j = th * 4 + jh
            nc.sync.dma_start(
                xa_in[ds(j, 1), :, :, :, :, :],
                xv[jh * 32 : (jh + 1) * 32, ds(th, 1), :, :, :],
            )


def _a2a_t2s(nc, tc, x_sb, xa_in, xa_out):
    _send_t2s(nc, x_sb, xa_in)
    nc.gpsimd.collective_compute(
        "AllToAll", OP.bypass, replica_groups=RG,
        ins=[xa_in[:].opt()], outs=[xa_out[:].opt()],
    )
    # chunks (c) -> s-domain sbuf (32 DMAs)
    xo = xa_out[:].rearrange("c sll slh tl b d -> c sll tl slh b d")
    for sll in range(4):
        for c in range(NC):
            nc.sync.dma_start(
                x_sb[sll * 32 + c * 4 : sll * 32 + c * 4 + 4, :, :],
                xo[ds(c, 1), ds(sll, 1), :, :, :, :],
            )


def _a2a_s2t(nc, tc, x_sb, xa_in, xa_out):
    # s-domain sbuf -> bounce chunks (32 DMAs)
    xi = xa_in[:].rearrange("j sll slh tl b d -> j sll tl slh b d")
    for j in range(NC):
        for sll in range(4):
            nc.sync.dma_start(
                xi[ds(j, 1), ds(sll, 1), :, :, :, :],
                x_sb[sll * 32 + j * 4 : sll * 32 + j * 4 + 4, :, :],
            )
    nc.gpsimd.collective_compute(
        "AllToAll", OP.bypass, replica_groups=RG,
        ins=[xa_in[:].opt()], outs=[xa_out[:].opt()],
    )
    # chunks (cs = th*4+jh) -> t-domain sbuf (8 DMAs)
    xv = x_sb[:].rearrange("p (th tl b) d -> p th tl b d", th=2, tl=4)
    for cs in range(NC):
        th, jh = cs // 4, cs % 4
        nc.sync.dma_start(
            xv[jh * 32 : (jh + 1) * 32, ds(th, 1), :, :, :],
            xa_out[ds(cs, 1), :, :, :, :, :],
        )


# ---------------------------------------------------------------------------
# pre: rv = rmsnorm(tokens) @ vrW  (t-domain) + AllToAll to s-domain
# ---------------------------------------------------------------------------
def _pre_rv(nc, tc, x_sb, ident, vrW, rv_t_d, rv_ain, rv_s_d):
    with ExitStack() as ctx:
        wp = ctx.enter_context(tc.tile_pool(name="vrw", bufs=1))
        vw = wp.tile([128, KT, 768], F32R, name="vw")
        nc.sync.dma_start(vw[:], vrW[:].rearrange("(kt p) m -> p kt m", p=128))

        rvp = ctx.enter_context(tc.tile_pool(name="rvp", bufs=1))
        rv_sb = rvp.tile([128, NT, 768], F32, name="rv_sb")
        sp = ctx.enter_context(tc.tile_pool(name="prsp", bufs=1))
        np_ = ctx.enter_context(tc.tile_pool(name="prnp", bufs=2))
        ps_tr = ctx.enter_context(
            tc.tile_pool(name="prps_tr", bufs=2, space="PSUM")
        )
        ps_pj = ctx.enter_context(
            tc.tile_pool(name="prps_pj", bufs=2, space="PSUM")
        )

        for sv in range(8):
            sq = sp.tile([128, 768], F32, name="prsq", tag="prsq")
            ss = np_.tile([128, 2], F32, name="prss", tag="prss")
            for j in range(2):
                nc.scalar.activation(
                    sq[:], x_sb[:, ds(sv + 8 * j, 1), :].squeeze(1), AF.Square,
                    accum_out=ss[:, j : j + 1],
                )
            inv = np_.tile([128, 2], F32, name="prinv", tag="prinv")
            _emit_rsqrt(nc, np_, inv[:], ss[:], 1.0 / 768.0, 1e-6, 1e-30)
            tn_t = sp.tile([128, 2, 768], F32R, name="prtn", tag="prtn")
            for j in range(2):
                nc.vector.tensor_scalar_mul(
                    tn_t[:, j, :], x_sb[:, ds(sv + 8 * j, 1), :].squeeze(1),
                    inv[:, j : j + 1],
                )
            tn_f = sp.tile([128, KT, 256], F32R, name="prtf", tag="prtf")
            for kt in range(KT):
                pt = ps_tr.tile([128, 256], F32R, name="prpt", tag="prps_tr")
                for j in range(2):
                    nc.tensor.transpose(
                        pt[:, j * 128 : (j + 1) * 128],
                        tn_t[:, j, kt * 128 : (kt + 1) * 128],
                        ident[:],
                    )
                nc.scalar.copy(tn_f[:, kt, :], pt[:].bitcast(F32))
            for j in range(2):
                for nh in range(2):
                    pv = ps_pj.tile([128, 384], F32, name="prpv", tag="prps_pj")
                    for kt in range(KT):
                        nc.tensor.matmul(
                            pv[:],
                            lhsT=tn_f[:, kt, j * 128 : (j + 1) * 128],
                            rhs=vw[:, kt, nh * 384 : (nh + 1) * 384],
                            start=(kt == 0),
                            stop=(kt == KT - 1),
                        )
                    nc.scalar.copy(
                        rv_sb[:, sv + 8 * j, nh * 384 : (nh + 1) * 384], pv[:]
                    )

        nc.sync.dma_start(
            rv_t_d[:].rearrange("(t p) d -> p t d", p=128), rv_sb[:]
        )
        _send_t2s(nc, rv_sb, rv_ain)
        nc.gpsimd.collective_compute(
            "AllToAll", OP.bypass, replica_groups=RG,
            ins=[rv_ain[:].opt()], outs=[rv_s_d[:].opt()],
        )


# ---------------------------------------------------------------------------
# space attention layer (t-domain; 8 seqs of 256 tokens)
# ---------------------------------------------------------------------------
def _attn_layer(nc, tc, L, x_sb, ident, rv_in, Wq3, Wk3, Wv3, Wo3, Wmg3, kg3):
    with ExitStack() as ctx:
        wp = ctx.enter_context(tc.tile_pool(name=f"wq{L}", bufs=1))
        wq = wp.tile([128, KT, 768], F32R, name=f"wq_t{L}")
        wk = wp.tile([128, KT, 768], F32R, name=f"wk_t{L}")
        wv = wp.tile([128, KT, 768], F32R, name=f"wv_t{L}")
        wo = wp.tile([128, KT, 768], F32R, name=f"wo_t{L}")
        wmg = wp.tile([128, KT, 24], F32R, name=f"wmg_t{L}")
        kgbc = wp.tile([128, 768], F32, name=f"kgbc{L}")
        for w_t, W in ((wq, Wq3), (wk, Wk3), (wv, Wv3), (wo, Wo3), (wmg, Wmg3)):
            nc.sync.dma_start(
                w_t[:], W[L].rearrange("(kt p) m -> p kt m", p=128)
            )
        nc.sync.dma_start(kgbc[:], kg3[L : L + 1, :].partition_broadcast(128))

        sp = ctx.enter_context(tc.tile_pool(name=f"sp{L}", bufs=1))
        sp2 = ctx.enter_context(tc.tile_pool(name=f"sp2{L}", bufs=2))
        hp = ctx.enter_context(tc.tile_pool(name=f"hp{L}", bufs=3))
        np_ = ctx.enter_context(tc.tile_pool(name=f"np{L}", bufs=2))
        ps_tr = ctx.enter_context(
            tc.tile_pool(name=f"ps_tr{L}", bufs=2, space="PSUM")
        )
        ps_pj = ctx.enter_context(
            tc.tile_pool(name=f"ps_pj{L}", bufs=2, space="PSUM")
        )
        ps_S = ctx.enter_context(
            tc.tile_pool(name=f"ps_S{L}", bufs=2, space="PSUM")
        )
        ps_O = ctx.enter_context(
            tc.tile_pool(name=f"ps_O{L}", bufs=2, space="PSUM")
        )

        rv_tv = rv_in[:].rearrange("(th r p) d -> p th r d", th=2, p=128)

        def seq_body(sv):
            # ---- rv slice for this seq (tiles sv, sv+8)
            rv_sl = sp.tile([128, 2, 768], F32, name="rv_sl", tag="rv_sl")
            nc.sync.dma_start(rv_sl[:], rv_tv[:, :, ds(sv, 1), :])
            # ---- rmsnorm
            sq = sp.tile([128, 768], F32, name="sq", tag="sq")
            ss = np_.tile([128, 2], F32, name="ss", tag="ss")
            for j in range(2):
                nc.scalar.activation(
                    sq[:], x_sb[:, ds(sv + 8 * j, 1), :].squeeze(1), AF.Square,
                    accum_out=ss[:, j : j + 1],
                )
            inv = np_.tile([128, 2], F32, name="inv", tag="inv")
            _emit_rsqrt(nc, np_, inv[:], ss[:], 1.0 / 768.0, 1e-6, 1e-30)
            tn_t = sp.tile([128, 2, 768], F32R, name="tn_t", tag="tn_t")
            for j in range(2):
                nc.vector.tensor_scalar_mul(
                    tn_t[:, j, :], x_sb[:, ds(sv + 8 * j, 1), :].squeeze(1),
                    inv[:, j : j + 1],
                )
            # ---- transpose tn -> tn_f
            tn_f = sp.tile([128, KT, 256], F32R, name="tn_f", tag="tn_f")
            for kt in range(KT):
                pt = ps_tr.tile([128, 256], F32R, name="pt_tn", tag="ps_tr")
                for j in range(2):
                    nc.tensor.transpose(
                        pt[:, j * 128 : (j + 1) * 128],
                        tn_t[:, j, kt * 128 : (kt + 1) * 128],
                        ident[:],
                    )
                nc.scalar.copy(tn_f[:, kt, :], pt[:].bitcast(F32))
            # ---- q projection (feature-major)
            q_f = sp2.tile([128, KT, 256], F32R, name="q_f", tag="q_f")
            for m in range(KT):
                pq = ps_pj.tile([128, 384], F32, name="pq", tag="ps_pj")
                for kt in range(KT):
                    nc.tensor.matmul(
                        pq[:, :256],
                        lhsT=wq[:, kt, m * 128 : (m + 1) * 128],
                        rhs=tn_f[:, kt, :],
                        start=(kt == 0),
                        stop=(kt == KT - 1),
                    )
                nc.scalar.copy(q_f[:, m, :], pq[:, :256])
            # ---- k projection (token-major) + l2norm * kgamma
            kraw = sp.tile([128, 2, 768], F32R, name="kraw", tag="kraw")
            for j in range(2):
                for nh in range(2):
                    pk = ps_pj.tile([128, 384], F32, name="pk", tag="ps_pj")
                    for kt in range(KT):
                        nc.tensor.matmul(
                            pk[:],
                            lhsT=tn_f[:, kt, j * 128 : (j + 1) * 128],
                            rhs=wk[:, kt, nh * 384 : (nh + 1) * 384],
                            start=(kt == 0),
                            stop=(kt == KT - 1),
                        )
                    nc.scalar.copy(kraw[:, j, nh * 384 : (nh + 1) * 384], pk[:])
            kss = np_.tile([128, 24], F32, name="kss", tag="kss")
            for j in range(2):
                nc.vector.tensor_tensor(
                    sq[:], kraw[:, j, :].bitcast(F32),
                    kraw[:, j, :].bitcast(F32), op=OP.mult
                )
                nc.vector.tensor_reduce(
                    out=kss[:, j * 12 : (j + 1) * 12],
                    in_=sq[:].rearrange("p (h d) -> p h d", h=H),
                    axis=mybir.AxisListType.X,
                    op=OP.add,
                )
            kinv = np_.tile([128, 24], F32, name="kinv", tag="kinv")
            _emit_rsqrt(nc, np_, kinv[:], kss[:], 1.0, 0.0, 1e-24)
            kib = sp.tile([128, 768], F32, name="kib", tag="kib")
            for j in range(2):
                nc.vector.tensor_copy(
                    kib[:].rearrange("p (h d) -> p h d", h=H),
                    kinv[:, j * 12 : (j + 1) * 12]
                    .unsqueeze(2)
                    .broadcast_to([128, H, DH]),
                )
                nc.vector.tensor_tensor(kib[:], kib[:], kgbc[:], op=OP.mult)
                nc.vector.tensor_tensor(
                    kraw[:, j, :], kraw[:, j, :].bitcast(F32), kib[:],
                    op=OP.mult,
                )
            k_f = sp2.tile([128, KT, 256], F32R, name="k_f", tag="k_f")
            for kt in range(KT):
                pt = ps_tr.tile([128, 256], F32R, name="pt_k", tag="ps_tr")
                for j in range(2):
                    nc.tensor.transpose(
                        pt[:, j * 128 : (j + 1) * 128],
                        kraw[:, j, kt * 128 : (kt + 1) * 128],
                        ident[:],
                    )
                nc.scalar.copy(k_f[:, kt, :], pt[:].bitcast(F32))
            # ---- mix / gates (sigmoid via tanh)
            mgs = np_.tile([128, 2, 24], F32, name="mgs", tag="mgs")
            for j in range(2):
                pm = ps_O.tile([128, 65], F32, name="pm", tag="ps_O")
                for kt in range(KT):
                    nc.tensor.matmul(
                        pm[:, :24],
                        lhsT=tn_f[:, kt, j * 128 : (j + 1) * 128],
                        rhs=wmg[:, kt, :],
                        start=(kt == 0),
                        stop=(kt == KT - 1),
                    )
                nc.scalar.activation(mgs[:, j, :], pm[:, :24], AF.Tanh, scale=0.5)
            nc.vector.tensor_scalar(
                mgs[:], mgs[:], 0.5, 0.5, op0=OP.mult, op1=OP.add
            )
            # ---- v projection + value-residual lerp -> v1 (bf16, |1 col)
            v1 = sp2.tile([128, 2, H, 65], BF16, name="v1", tag="v1")
            mixb = kib
            tdt = sq[:, 0:384]
            for j in range(2):
                nc.vector.tensor_copy(
                    mixb[:].rearrange("p (h d) -> p h d", h=H),
                    mgs[:, j, 0:12].unsqueeze(2).broadcast_to([128, H, DH]),
                )
                for nh in range(2):
                    pv = ps_pj.tile([128, 384], F32, name="pv", tag="ps_pj")
                    for kt in range(KT):
                        nc.tensor.matmul(
                            pv[:],
                            lhsT=tn_f[:, kt, j * 128 : (j + 1) * 128],
                            rhs=wv[:, kt, nh * 384 : (nh + 1) * 384],
                            start=(kt == 0),
                            stop=(kt == KT - 1),
                        )
                    nc.vector.tensor_tensor(
                        tdt, rv_sl[:, j, nh * 384 : (nh + 1) * 384], pv[:],
                        op=OP.subtract,
                    )
                    nc.vector.tensor_tensor(
                        tdt, tdt, mixb[:, nh * 384 : (nh + 1) * 384],
                        op=OP.mult,
                    )
                    nc.vector.tensor_tensor(
                        v1[:, j, 6 * nh : 6 * nh + 6, 0:64],
                        pv[:].rearrange("p (h d) -> p h d", h=6),
                        tdt.rearrange("p (h d) -> p h d", h=6),
                        op=OP.add,
                    )
                nc.vector.memset(v1[:, j, :, 64:65], 1.0)
            # ---- attention per head: scores k-major (no transposes)
            o_t = tn_t
            for h in range(H):
                pt_b = hp.tile([128, 2, 256], BF16, name="pt_b", tag="pt_b")
                st = hp.tile([128, 256], F32, name="st", tag="st")
                rec = np_.tile([128, 1], F32, name="rec", tag="rec")
                mt, po = h // 2, 64 * (h % 2)
                for kvt in range(2):
                    pS = ps_S.tile([128, 256], F32, name="pS", tag="ps_S")
                    nc.tensor.matmul(
                        pS[:],
                        lhsT=k_f[po : po + 64, mt, kvt * 128 : (kvt + 1) * 128],
                        rhs=q_f[po : po + 64, mt, :],
                        start=True,
                        stop=True,
                    )
                    nc.scalar.activation(st[:], pS[:], AF.Tanh)
                    nc.scalar.activation(pt_b[:, kvt, :], st[:], AF.Exp, scale=50.0)
                for qt in range(2):
                    pO = ps_O.tile([128, 65], F32, name="pO", tag="ps_O")
                    for kvt in range(2):
                        nc.tensor.matmul(
                            pO[:],
                            lhsT=pt_b[:, kvt, qt * 128 : (qt + 1) * 128],
                            rhs=v1[:, kvt, h, :],
                            start=(kvt == 0),
                            stop=(kvt == 1),
                        )
                    nc.vector.reciprocal(rec[:], pO[:, 64:65])
                    nc.vector.tensor_tensor(
                        rec[:], rec[:], mgs[:, qt, 12 + h : 13 + h], op=OP.mult
                    )
                    nc.vector.tensor_scalar_mul(
                        o_t[:, qt, 64 * h : 64 * h + 64], pO[:, 0:64], rec[:]
                    )
            # ---- transpose o -> o_f, then Wo and residual add
            o_f = tn_f
            for kt in range(KT):
                pt = ps_tr.tile([128, 256], F32R, name="pt_o", tag="ps_tr")
                for j in range(2):
                    nc.tensor.transpose(
                        pt[:, j * 128 : (j + 1) * 128],
                        o_t[:, j, kt * 128 : (kt + 1) * 128],
                        ident[:],
                    )
                nc.scalar.copy(o_f[:, kt, :], pt[:].bitcast(F32))
            for j in range(2):
                for nh in range(2):
                    px = ps_pj.tile([128, 384], F32, name="px", tag="ps_pj")
                    for kt in range(KT):
                        nc.tensor.matmul(
                            px[:],
                            lhsT=o_f[:, kt, j * 128 : (j + 1) * 128],
                            rhs=wo[:, kt, nh * 384 : (nh + 1) * 384],
                            start=(kt == 0),
                            stop=(kt == KT - 1),
                        )
                    xs = x_sb[:, ds(sv + 8 * j, 1), nh * 384 : (nh + 1) * 384]
                    xs = xs.squeeze(1)
                    nc.vector.tensor_tensor(xs, xs, px[:], op=OP.add)

        for _sv in range(8):
            seq_body(_sv)


# ---------------------------------------------------------------------------
# time attention layer (s-domain; 8 groups of 2 tiles; 4 causal seqs of 32
# per 128-token tile, rotary + block-diag causal mask)
# ---------------------------------------------------------------------------
def _time_layer(nc, tc, L, x_sb, ident, pmat, rotc, rots, mask_b, rv_view,
                Wq2, Wk2, Wv2, Wo2, Wmg2, kg2):
    with ExitStack() as ctx:
        wp = ctx.enter_context(tc.tile_pool(name=f"twq{L}", bufs=1))
        wq = wp.tile([128, KT, 768], F32R, name=f"twq_t{L}")
        wk = wp.tile([128, KT, 768], F32R, name=f"twk_t{L}")
        wv = wp.tile([128, KT, 768], F32R, name=f"twv_t{L}")
        wo = wp.tile([128, KT, 768], F32R, name=f"two_t{L}")
        wmg = wp.tile([128, KT, 24], F32R, name=f"twmg_t{L}")
        kgbc = wp.tile([128, 768], F32, name=f"tkgbc{L}")
        for w_t, W in ((wq, Wq2), (wk, Wk2), (wv, Wv2), (wo, Wo2), (wmg, Wmg2)):
            nc.sync.dma_start(
                w_t[:], W[L].rearrange("(kt p) m -> p kt m", p=128)
            )
        nc.sync.dma_start(kgbc[:], kg2[L : L + 1, :].partition_broadcast(128))

        sp = ctx.enter_context(tc.tile_pool(name=f"tsp{L}", bufs=1))
        sp2 = ctx.enter_context(tc.tile_pool(name=f"tsp2{L}", bufs=2))
        hp = ctx.enter_context(tc.tile_pool(name=f"thp{L}", bufs=3))
        np_ = ctx.enter_context(tc.tile_pool(name=f"tnp{L}", bufs=2))
        ps_tr = ctx.enter_context(
            tc.tile_pool(name=f"tps_tr{L}", bufs=2, space="PSUM")
        )
        ps_pj = ctx.enter_context(
            tc.tile_pool(name=f"tps_pj{L}", bufs=2, space="PSUM")
        )
        ps_S = ctx.enter_context(
            tc.tile_pool(name=f"tps_S{L}", bufs=2, space="PSUM")
        )
        ps_O = ctx.enter_context(
            tc.tile_pool(name=f"tps_O{L}", bufs=2, space="PSUM")
        )

        def grp_body(gv):
            off = gv * 2
            rv_sl = sp.tile([128, 2, 768], F32, name="trv", tag="trv")
            for sll in range(4):
                nc.sync.dma_start(
                    rv_sl[sll * 32 : (sll + 1) * 32, :, :],
                    rv_view[ds(gv, 1), ds(sll, 1), :, :, :, :],
                )
            # ---- rmsnorm
            sq = sp.tile([128, 768], F32, name="tsq", tag="tsq")
            ss = np_.tile([128, 2], F32, name="tss", tag="tss")
            for j in range(2):
                nc.scalar.activation(
                    sq[:], x_sb[:, ds(off + j, 1), :].squeeze(1), AF.Square,
                    accum_out=ss[:, j : j + 1],
                )
            inv = np_.tile([128, 2], F32, name="tinv", tag="tinv")
            _emit_rsqrt(nc, np_, inv[:], ss[:], 1.0 / 768.0, 1e-6, 1e-30)
            tn_t = sp.tile([128, 2, 768], F32R, name="ttn", tag="ttn")
            for j in range(2):
                nc.vector.tensor_scalar_mul(
                    tn_t[:, j, :], x_sb[:, ds(off + j, 1), :].squeeze(1),
                    inv[:, j : j + 1],
                )
            # ---- transpose tn -> tn_f
            tn_f = sp.tile([128, KT, 256], F32R, name="ttf", tag="ttf")
            for kt in range(KT):
                pt = ps_tr.tile([128, 256], F32R, name="tpt", tag="tps_tr")
                for j in range(2):
                    nc.tensor.transpose(
                        pt[:, j * 128 : (j + 1) * 128],
                        tn_t[:, j, kt * 128 : (kt + 1) * 128],
                        ident[:],
                    )
                nc.scalar.copy(tn_f[:, kt, :], pt[:].bitcast(F32))
            # ---- q projection (feature-major) + rotary -> q_r (bf16)
            q_f = sp2.tile([128, KT, 256], F32R, name="tq_f", tag="tq_f")
            for m in range(KT):
                pq = ps_pj.tile([128, 384], F32, name="tpq", tag="tps_pj")
                for kt in range(KT):
                    nc.tensor.matmul(
                        pq[:, :256],
                        lhsT=wq[:, kt, m * 128 : (m + 1) * 128],
                        rhs=tn_f[:, kt, :],
                        start=(kt == 0),
                        stop=(kt == KT - 1),
                    )
                nc.scalar.copy(q_f[:, m, :], pq[:, :256])
            q_r = sp2.tile([128, KT, 256], BF16, name="tq_r", tag="tq_r")
            t1 = sp.tile([128, 256], F32, name="trt1", tag="trt1")
            t2 = sp.tile([128, 256], F32, name="trt2", tag="trt2")
            for m in range(KT):
                pr = ps_tr.tile([128, 256], F32, name="tpr", tag="tps_tr")
                nc.tensor.matmul(
                    pr[:], lhsT=pmat[:], rhs=q_f[:, m, :],
                    start=True, stop=True,
                )
                nc.vector.tensor_tensor(
                    t1[:], q_f[:, m, :].bitcast(F32), rotc[:], op=OP.mult
                )
                nc.vector.tensor_tensor(t2[:], pr[:], rots[:], op=OP.mult)
                nc.vector.tensor_tensor(q_r[:, m, :], t1[:], t2[:], op=OP.add)
            # ---- k projection (token-major) + l2norm * kgamma
            kraw = sp.tile([128, 2, 768], F32R, name="tkraw", tag="tkraw")
            for j in range(2):
                for nh in range(2):
                    pk = ps_pj.tile([128, 384], F32, name="tpk", tag="tps_pj")
                    for kt in range(KT):
                        nc.tensor.matmul(
                            pk[:],
                            lhsT=tn_f[:, kt, j * 128 : (j + 1) * 128],
                            rhs=wk[:, kt, nh * 384 : (nh + 1) * 384],
                            start=(kt == 0),
                            stop=(kt == KT - 1),
                        )
                    nc.scalar.copy(kraw[:, j, nh * 384 : (nh + 1) * 384], pk[:])
            kss = np_.tile([128, 24], F32, name="tkss", tag="tkss")
            for j in range(2):
                nc.vector.tensor_tensor(
                    sq[:], kraw[:, j, :].bitcast(F32),
                    kraw[:, j, :].bitcast(F32), op=OP.mult
                )
                nc.vector.tensor_reduce(
                    out=kss[:, j * 12 : (j + 1) * 12],
                    in_=sq[:].rearrange("p (h d) -> p h d", h=H),
                    axis=mybir.AxisListType.X,
                    op=OP.add,
                )
            kinv = np_.tile([128, 24], F32, name="tkinv", tag="tkinv")
            _emit_rsqrt(nc, np_, kinv[:], kss[:], 1.0, 0.0, 1e-24)
            kib = sp.tile([128, 768], F32, name="tkib", tag="tkib")
            for j in range(2):
                nc.vector.tensor_copy(
                    kib[:].rearrange("p (h d) -> p h d", h=H),
                    kinv[:, j * 12 : (j + 1) * 12]
                    .unsqueeze(2)
                    .broadcast_to([128, H, DH]),
                )
                nc.vector.tensor_tensor(kib[:], kib[:], kgbc[:], op=OP.mult)
                nc.vector.tensor_tensor(
                    kraw[:, j, :], kraw[:, j, :].bitcast(F32), kib[:],
                    op=OP.mult,
                )
            # ---- transpose k -> k_f + rotary -> k_r (bf16)
            k_f = sp2.tile([128, KT, 256], F32R, name="tk_f", tag="tk_f")
            for kt in range(KT):
                pt = ps_tr.tile([128, 256], F32R, name="tptk", tag="tps_tr")
                for j in range(2):
                    nc.tensor.transpose(
                        pt[:, j * 128 : (j + 1) * 128],
                        kraw[:, j, kt * 128 : (kt + 1) * 128],
                        ident[:],
                    )
                nc.scalar.copy(k_f[:, kt, :], pt[:].bitcast(F32))
            k_r = sp2.tile([128, KT, 256], BF16, name="tk_r", tag="tk_r")
            for m in range(KT):
                pr = ps_tr.tile([128, 256], F32, name="tprk", tag="tps_tr")
                nc.tensor.matmul(
                    pr[:], lhsT=pmat[:], rhs=k_f[:, m, :],
                    start=True, stop=True,
                )
                nc.vector.tensor_tensor(
                    t1[:], k_f[:, m, :].bitcast(F32), rotc[:], op=OP.mult
                )
                nc.vector.tensor_tensor(t2[:], pr[:], rots[:], op=OP.mult)
                nc.vector.tensor_tensor(k_r[:, m, :], t1[:], t2[:], op=OP.add)
            # ---- mix / gates
            mgs = np_.tile([128, 2, 24], F32, name="tmgs", tag="tmgs")
            for j in range(2):
                pm = ps_O.tile([128, 65], F32, name="tpm", tag="tps_O")
                for kt in range(KT):
                    nc.tensor.matmul(
                        pm[:, :24],
                        lhsT=tn_f[:, kt, j * 128 : (j + 1) * 128],
                        rhs=wmg[:, kt, :],
                        start=(kt == 0),
                        stop=(kt == KT - 1),
                    )
                nc.scalar.activation(mgs[:, j, :], pm[:, :24], AF.Tanh, scale=0.5)
            nc.vector.tensor_scalar(
                mgs[:], mgs[:], 0.5, 0.5, op0=OP.mult, op1=OP.add
            )
            # ---- v projection + value-residual lerp -> v1
            v1 = sp2.tile([128, 2, H, 65], BF16, name="tv1", tag="tv1")
            mixb = kib
            tdt = sq[:, 0:384]
            for j in range(2):
                nc.vector.tensor_copy(
                    mixb[:].rearrange("p (h d) -> p h d", h=H),
                    mgs[:, j, 0:12].unsqueeze(2).broadcast_to([128, H, DH]),
                )
                for nh in range(2):
                    pv = ps_pj.tile([128, 384], F32, name="tpv", tag="tps_pj")
                    for kt in range(KT):
                        nc.tensor.matmul(
                            pv[:],
                            lhsT=tn_f[:, kt, j * 128 : (j + 1) * 128],
                            rhs=wv[:, kt, nh * 384 : (nh + 1) * 384],
                            start=(kt == 0),
                            stop=(kt == KT - 1),
                        )
                    nc.vector.tensor_tensor(
                        tdt, rv_sl[:, j, nh * 384 : (nh + 1) * 384], pv[:],
                        op=OP.subtract,
                    )
                    nc.vector.tensor_tensor(
                        tdt, tdt, mixb[:, nh * 384 : (nh + 1) * 384],
                        op=OP.mult,
                    )
                    nc.vector.tensor_tensor(
                        v1[:, j, 6 * nh : 6 * nh + 6, 0:64],
                        pv[:].rearrange("p (h d) -> p h d", h=6),
                        tdt.rearrange("p (h d) -> p h d", h=6),
                        op=OP.add,
                    )
                nc.vector.memset(v1[:, j, :, 64:65], 1.0)
            # ---- attention per (tile, head): causal via mask multiply
            o_t = tn_t
            for j in range(2):
                for h in range(H):
                    mt, po = h // 2, 64 * (h % 2)
                    pS = ps_S.tile([128, 128], F32, name="tpS", tag="tps_S")
                    nc.tensor.matmul(
                        pS[:],
                        lhsT=k_r[po : po + 64, mt, j * 128 : (j + 1) * 128],
                        rhs=q_r[po : po + 64, mt, j * 128 : (j + 1) * 128],
                        start=True,
                        stop=True,
                    )
                    st = hp.tile([128, 128], F32, name="tst", tag="tst")
                    nc.scalar.activation(st[:], pS[:], AF.Tanh)
                    eb = hp.tile([128, 128], BF16, name="teb", tag="teb")
                    nc.scalar.activation(eb[:], st[:], AF.Exp, scale=50.0)
                    me = hp.tile([128, 128], BF16, name="tme", tag="tme")
                    nc.vector.tensor_tensor(me[:], eb[:], mask_b[:], op=OP.mult)
                    pO = ps_O.tile([128, 65], F32, name="tpO", tag="tps_O")
                    nc.tensor.matmul(
                        pO[:], lhsT=me[:], rhs=v1[:, j, h, :],
                        start=True, stop=True,
                    )
                    rec = np_.tile([128, 1], F32, name="trec", tag="trec")
                    nc.vector.reciprocal(rec[:], pO[:, 64:65])
                    nc.vector.tensor_tensor(
                        rec[:], rec[:], mgs[:, j, 12 + h : 13 + h], op=OP.mult
                    )
                    nc.vector.tensor_scalar_mul(
                        o_t[:, j, 64 * h : 64 * h + 64], pO[:, 0:64], rec[:]
                    )
            # ---- transpose o -> o_f, then Wo and residual add
            o_f = tn_f
            for kt in range(KT):
                pt = ps_tr.tile([128, 256], F32R, name="tpto", tag="tps_tr")
                for j in range(2):
                    nc.tensor.transpose(
                        pt[:, j * 128 : (j + 1) * 128],
                        o_t[:, j, kt * 128 : (kt + 1) * 128],
                        ident[:],
                    )
                nc.scalar.copy(o_f[:, kt, :], pt[:].bitcast(F32))
            for j in range(2):
                for nh in range(2):
                    px = ps_pj.tile([128, 384], F32, name="tpx", tag="tps_pj")
                    for kt in range(KT):
                        nc.tensor.matmul(
                            px[:],
                            lhsT=o_f[:, kt, j * 128 : (j + 1) * 128],
                            rhs=wo[:, kt, nh * 384 : (nh + 1) * 384],
                            start=(kt == 0),
                            stop=(kt == KT - 1),
                        )
                    xs = x_sb[:, ds(off + j, 1), nh * 384 : (nh + 1) * 384]
                    xs = xs.squeeze(1)
                    nc.vector.tensor_tensor(xs, xs, px[:], op=OP.add)

        for _gv in range(8):
            grp_body(_gv)


def _ff_layer(nc, tc, L, x_sb, ident, Win3, Wout3, pfx):
    with ExitStack() as ctx:
        wop = ctx.enter_context(tc.tile_pool(name=f"{pfx}wop{L}", bufs=1))
        wout = wop.tile([128, 16, 768], F32R, name=f"{pfx}wout_t{L}")
        nc.sync.dma_start(
            wout[:], Wout3[L].rearrange("(kt p) m -> p kt m", p=128)
        )
        winp = ctx.enter_context(tc.tile_pool(name=f"{pfx}winp{L}", bufs=2))
        sp = ctx.enter_context(tc.tile_pool(name=f"{pfx}fsp{L}", bufs=1))
        up = ctx.enter_context(tc.tile_pool(name=f"{pfx}fup{L}", bufs=1))
        np_ = ctx.enter_context(tc.tile_pool(name=f"{pfx}fnp{L}", bufs=2))
        ps_tr = ctx.enter_context(
            tc.tile_pool(name=f"{pfx}fps_tr{L}", bufs=2, space="PSUM")
        )
        ps_h = ctx.enter_context(
            tc.tile_pool(name=f"{pfx}fps_h{L}", bufs=4, space="PSUM")
        )
        ps_xd = ctx.enter_context(
            tc.tile_pool(name=f"{pfx}fps_xd{L}", bufs=2, space="PSUM")
        )

        def chunk_body(cv):
            coff = cv * 4
            ss = np_.tile([128, 4], F32, name="ss2", tag="ss2")
            sq = sp.tile([128, 768], F32, name="fsq", tag="fsq")
            for j in range(4):
                nc.scalar.activation(
                    sq[:], x_sb[:, ds(coff + j, 1), :].squeeze(1), AF.Square,
                    accum_out=ss[:, j : j + 1],
                )
            inv = np_.tile([128, 4], F32, name="inv2", tag="inv2")
            _emit_rsqrt(nc, np_, inv[:], ss[:], 1.0 / 768.0, 1e-6, 1e-30)
            tn2 = sp.tile([128, 4, 768], F32R, name="tn2", tag="tn2")
            for j in range(4):
                nc.vector.tensor_scalar_mul(
                    tn2[:, j, :], x_sb[:, ds(coff + j, 1), :].squeeze(1),
                    inv[:, j : j + 1],
                )
            tn2f = sp.tile([128, KT, 512], F32R, name="tn2f", tag="tn2f")
            for kt in range(KT):
                pt = ps_tr.tile([128, 512], F32R, name="fpt", tag="fps_tr")
                for j in range(4):
                    nc.tensor.transpose(
                        pt[:, j * 128 : (j + 1) * 128],
                        tn2[:, j, kt * 128 : (kt + 1) * 128],
                        ident[:],
                    )
                nc.scalar.copy(tn2f[:, kt, :], pt[:].bitcast(F32))
            # ---- h = tn2 @ Win; u = a * gelu(g)
            u = up.tile([128, 16, 512], F32R, name="u", tag="u")
            gl = sp.tile([128, 512], F32, name="gl", tag="gl")
            for m in range(16):
                wa = winp.tile([128, KT, 128], F32R, name="wa", tag="wa")
                wg = winp.tile([128, KT, 128], F32R, name="wg", tag="wg")
                nc.sync.dma_start(
                    wa[:],
                    Win3[L, :, m * 128 : (m + 1) * 128].rearrange(
                        "(kt p) m -> p kt m", p=128
                    ),
                )
                nc.sync.dma_start(
                    wg[:],
                    Win3[L, :, 2048 + m * 128 : 2048 + (m + 1) * 128].rearrange(
                        "(kt p) m -> p kt m", p=128
                    ),
                )
                pa = ps_h.tile([128, 512], F32, name="pa", tag="fps_h")
                pg = ps_h.tile([128, 512], F32, name="pg", tag="fps_h")
                for kt in range(KT):
                    nc.tensor.matmul(
                        pa[:], lhsT=wa[:, kt, :], rhs=tn2f[:, kt, :],
                        start=(kt == 0), stop=(kt == KT - 1),
                    )
                for kt in range(KT):
                    nc.tensor.matmul(
                        pg[:], lhsT=wg[:, kt, :], rhs=tn2f[:, kt, :],
                        start=(kt == 0), stop=(kt == KT - 1),
                    )
                nc.scalar.activation(gl[:], pg[:], AF.Gelu)
                nc.vector.tensor_tensor(u[:, m, :], pa[:], gl[:], op=OP.mult)
            # ---- x += u @ Wout
            for j in range(4):
                for nh in range(2):
                    px = ps_xd.tile([128, 384], F32, name="fpx", tag="fps_xd")
                    for ktf in range(16):
                        nc.tensor.matmul(
                            px[:],
                            lhsT=u[:, ktf, j * 128 : (j + 1) * 128],
                            rhs=wout[:, ktf, nh * 384 : (nh + 1) * 384],
                            start=(ktf == 0),
                            stop=(ktf == 15),
                        )
                    xs = x_sb[:, ds(coff + j, 1), nh * 384 : (nh + 1) * 384]
                    xs = xs.squeeze(1)
                    nc.vector.tensor_tensor(xs, xs, px[:], op=OP.add)

        for _cv in range(4):
            chunk_body(_cv)


def _final_norm(nc, tc, x_sb, x_out):
    xo_view = x_out[:].rearrange("(t p) d -> p t d", p=128)
    with ExitStack() as ctx:
        sp = ctx.enter_context(tc.tile_pool(name="fino", bufs=2))
        np_ = ctx.enter_context(tc.tile_pool(name="finn", bufs=2))
        for cv in range(4):
            coff = cv * 4
            sq = sp.tile([128, 768], F32, name="gsq", tag="gsq")
            ss = np_.tile([128, 4], F32, name="gss", tag="gss")
            for j in range(4):
                nc.scalar.activation(
                    sq[:], x_sb[:, ds(coff + j, 1), :].squeeze(1), AF.Square,
                    accum_out=ss[:, j : j + 1],
                )
            inv = np_.tile([128, 4], F32, name="ginv", tag="ginv")
            _emit_rsqrt(nc, np_, inv[:], ss[:], 1.0 / 768.0, 1e-6, 1e-30)
            ot = sp.tile([128, 4, 768], F32, name="got", tag="got")
            for j in range(4):
                nc.vector.tensor_scalar_mul(
                    ot[:, j, :], x_sb[:, ds(coff + j, 1), :].squeeze(1),
                    inv[:, j : j + 1],
                )
            nc.sync.dma_start(xo_view[:, ds(coff, 4), :], ot[:])


# revision 12
# speedup vs baseline: 2.7819x; 2.7819x over previous
"""AxialSpaceTimeTransformer on 8 TRN2 NeuronCores — single full Bass kernel.

Sharding (8-way, single chip):
  * t-domain: core c holds frames t in [4c, 4c+4) for both batches.
    Space-attention (over s) and FF are core-local here.
  * s-domain: core c holds spatial positions s in [32c, 32c+32).
    Causal time-attention (over t) is core-local here.

The ENTIRE network (rv projection, 6 space layers, 2 causal time layers
with rotary, all FFs, final rmsnorm) runs as ONE Bass kernel invoked once
per call; the four t<->s reshardings are in-kernel AllToAll collectives
through DRAM bounce buffers.  No XLA compute stages remain.
"""

import os
import sys
import types

import numpy as np

if "/opt/trn_rl_repo" not in sys.path:
    sys.path.insert(0, "/opt/trn_rl_repo")

# -- antenv.axon_hooks shim (agent image lacks it; bass_utils wants it) --
import antenv  # noqa: E402

if not hasattr(antenv, "axon_hooks"):
    _hooks = types.ModuleType("antenv.axon_hooks")
    _hooks._hook = None
    _hooks.set_axon_ntff_profile_hook = lambda h: setattr(_hooks, "_hook", h)
    _hooks.get_axon_ntff_profile_hook = lambda: _hooks._hook
    sys.modules["antenv.axon_hooks"] = _hooks
    antenv.axon_hooks = _hooks
    try:
        from trn_agent_boot.trn_boot import _ntff_profile_via_ctypes

        _hooks.set_axon_ntff_profile_hook(
            _ntff_profile_via_ctypes("/opt/axon/libaxon_pjrt.so")
        )
    except Exception:
        pass

import jax  # noqa: E402
import jax.numpy as jnp  # noqa: E402
from jax.sharding import Mesh, NamedSharding, PartitionSpec as P  # noqa: E402
from jax.experimental.shard_map import shard_map  # noqa: E402

DIM = 768
DEPTH = 8
HEADS = 12
DH = 64
DFF = 2048
SOFTCLAMP = 50.0
B, T, S = 2, 32, 256
EPS = 1e-6
NC = 8
TL = T // NC  # 4 frames/core (t-domain)
SL = S // NC  # 32 positions/core (s-domain)
NTOK = B * TL * S  # 2048 tokens per core in either domain


def _round_f32r(x):
    """fp32 -> fp32r (13 explicit mantissa bits, RNE) rounding on host."""
    u = np.ascontiguousarray(x, dtype=np.float32).view(np.uint32)
    lsb = (u >> 10) & 1
    r = (u + 0x1FF + lsb) & np.uint32(0xFFFFFC00)
    return r.view(np.float32).copy()


def _make_rotary(n):
    inv = 1.0 / (10000.0 ** (np.arange(0, DH, 2, dtype=np.float32) / DH))
    f = np.arange(n, dtype=np.float32)[:, None] * inv[None, :]
    return np.concatenate([f, f], axis=-1)  # (n, DH)


def _pack_inputs(inputs):
    """Host-side weight folding/packing for the bass kernel (np arrays)."""
    f32 = np.float32
    SP = [0, 1, 2, 4, 5, 6]
    TM = [3, 7]
    anw = np.asarray(inputs["attn_norm_w"], f32)[:, :, None]
    fnw = np.asarray(inputs["ff_norm_w"], f32)[:, :, None]
    Wq = np.asarray(inputs["Wq"], f32) * anw
    Wk = np.asarray(inputs["Wk"], f32) * anw
    Wv = np.asarray(inputs["Wv"], f32) * anw
    Wo = np.asarray(inputs["Wo"], f32)
    Wmg = np.concatenate(
        [
            np.asarray(inputs["Wmix"], f32) * anw,
            np.asarray(inputs["Wg"], f32) * anw,
        ],
        axis=2,
    )  # (8, 768, 24)
    # k scale applied after l2norm; folds sqrt(DH), 1/sqrt(DH) and 1/softclamp
    kg = ((np.asarray(inputs["k_gamma"], f32) + 1.0) / SOFTCLAMP).reshape(8, 768)
    Win = np.asarray(inputs["Win"], f32) * fnw
    Wout = np.asarray(inputs["Wout"], f32)

    g = {
        "Wq6": _round_f32r(Wq[SP]),
        "Wk6": _round_f32r(Wk[SP]),
        "Wv6": _round_f32r(Wv[SP]),
        "Wo6": _round_f32r(Wo[SP]),
        "Wmg6": _round_f32r(Wmg[SP]),
        "kg6": kg[SP].astype(f32),
        "Win6": _round_f32r(Win[SP]),
        "Wout6": _round_f32r(Wout[SP]),
        "WqT": _round_f32r(Wq[TM]),
        "WkT": _round_f32r(Wk[TM]),
        "WvT": _round_f32r(Wv[TM]),
        "WoT": _round_f32r(Wo[TM]),
        "WmgT": _round_f32r(Wmg[TM]),
        "kgT": kg[TM].astype(f32),
        "WinT": _round_f32r(Win[TM]),
        "WoutT": _round_f32r(Wout[TM]),
        "vrW": _round_f32r(
            np.asarray(inputs["vr_norm_w"], f32)[:, None]
            * np.asarray(inputs["vr_W"], f32)
        ),
    }
    # rotary tables, feature-major: [p in 0..128 = 2 heads x 64 d, n in 0..256]
    rot = _make_rotary(T)  # (32, 64)
    pp = np.arange(128)[:, None] % 64
    nn = np.arange(256)[None, :] % T
    g["rotc"] = np.cos(rot[nn, pp]).astype(f32)
    g["rots"] = np.sin(rot[nn, pp]).astype(f32)
    # rotate-half permutation as a matmul stationary: Pq = pmat.T @ q_f
    pm = np.zeros((128, 128), f32)
    for i in range(128):
        base, d = (i // 64) * 64, i % 64
        if d < 32:
            pm[base + d + 32, i] = -1.0
        else:
            pm[base + d - 32, i] = 1.0
    g["pmat"] = pm
    # block-diag causal mask [k-part, q-free] over 4 seqs of 32
    kp = np.arange(128)[:, None]
    qc = np.arange(128)[None, :]
    g["maskf"] = (
        ((kp // 32 == qc // 32) & (kp % 32 <= qc % 32)).astype(f32)
    )
    return g


# ---------------------------------------------------------------------------
# cached compiled pipeline
# ---------------------------------------------------------------------------
_PIPE = None


def _build_pipeline(inputs):
    devs = jax.devices()[:NC]
    mesh = Mesh(np.asarray(devs), ("core",))
    shard = NamedSharding(mesh, P("core"))

    nc, in_names, out_names, out_avals = build_full()
    from concourse import bass2jax
    from concourse.bass2jax import _bass_exec_p

    bind_names = tuple(in_names + out_names)
    pid_name = nc.partition_id_tensor.name if nc.partition_id_tensor else None
    full_names = bind_names + ((pid_name,) if pid_name else ())

    def bass_body(*args):
        ops = list(args)
        if pid_name is not None:
            ops.append(bass2jax.partition_id_tensor())
        outs = _bass_exec_p.bind(
            *ops,
            out_avals=tuple(out_avals),
            in_names=full_names,
            out_names=tuple(out_names),
            lowering_input_output_aliases=(),
            sim_require_finite=True,
            sim_require_nnan=True,
            nc=nc,
        )
        return tuple(outs)

    percore = {"x_in", "x_out"}
    in_specs = tuple(P("core") if n in percore else P() for n in bind_names)
    out_specs = (P("core"),) * len(out_names)
    nout = len(out_names)
    bass_jit = jax.jit(
        shard_map(bass_body, mesh=mesh, in_specs=in_specs,
                  out_specs=out_specs, check_rep=False),
        donate_argnums=tuple(range(len(bind_names) - nout, len(bind_names))),
    )

    repl = NamedSharding(mesh, P())
    packs = {
        k: jax.device_put(jnp.asarray(v), repl)
        for k, v in _pack_inputs(inputs).items()
    }
    jax.block_until_ready(list(packs.values()))

    zjit = jax.jit(
        lambda: jnp.zeros((NC * NTOK, DIM), jnp.float32),
        out_shardings=shard,
    )

    def run(tok_flat):
        tok = jax.device_put(tok_flat, shard)
        ops = []
        for nme in in_names:
            if nme == "x_in":
                ops.append(tok)
            else:
                ops.append(packs[nme])
        (out,) = bass_jit(*ops, zjit())
        return out

    run.stages = {}
    return run


def shard_tokens(tokens):
    """(B,T,S,D) -> (NC*2048, D) t-domain rows: tile=(th,tl,b), p=(jh,sll,slh)."""
    A = tokens.reshape(B, NC, TL, 2, 4, 8, 4, DIM)  # b c tl th jh slh sll d
    A = A.transpose(1, 3, 2, 0, 4, 6, 5, 7)  # c th tl b jh sll slh d
    return np.ascontiguousarray(A).reshape(NC * NTOK, DIM)


def unshard_out(out):
    """(NC*2048, D) s-domain rows: tile=(slh,b), p=(sll,c,tl) -> (B,T,S,D)."""
    rec = out.reshape(NC, 8, 2, 4, 8, 4, DIM)  # core slh b sll c tl d
    rec = rec.transpose(2, 4, 5, 0, 1, 3, 6)  # b c tl core slh sll d
    return np.ascontiguousarray(rec).reshape(B, T, S, DIM)


def kernel(**inputs):
    global _PIPE
    tokens = np.asarray(inputs["tokens"], dtype=np.float32)
    tok_bt = shard_tokens(tokens)

    if _PIPE is None:
        _PIPE = _build_pipeline(inputs)
    out = np.asarray(jax.block_until_ready(_PIPE(jnp.asarray(tok_bt))))

    out = unshard_out(out)
    out = out * np.asarray(inputs["final_norm_w"], np.float32)
    return np.ascontiguousarray(out.astype(np.float32))


# ---------------------------------------------------------------------------
# Bass kernel
# ---------------------------------------------------------------------------
from contextlib import ExitStack  # noqa: E402

import concourse.bacc as bacc  # noqa: E402
import concourse.mybir as mybir  # noqa: E402
import concourse.tile as tile  # noqa: E402
from concourse.bass import ds  # noqa: E402
from concourse.masks import make_identity  # noqa: E402

F32 = mybir.dt.float32
F32R = mybir.dt.float32r
BF16 = mybir.dt.bfloat16
I32 = mybir.dt.int32
AF = mybir.ActivationFunctionType
OP = mybir.AluOpType

NT = 16  # token tiles (2048 tokens)
KT = 6  # 768 / 128 feature tiles
H = 12
RG = [list(range(NC))]


def _emit_rsqrt(nc, pool, out, in_, scale, bias, guard):
    """out = 1/sqrt(max(in_*scale + bias, guard)); quake seed + 3 Newton."""
    shp = [128, in_.shape[1]]
    m = pool.tile(shp, F32, name="rs_m", tag="rs_m")
    nc.vector.tensor_scalar(m[:], in_, scale, bias, op0=OP.mult, op1=OP.add)
    nc.vector.tensor_scalar_max(m[:], m[:], guard)
    yi = pool.tile(shp, I32, name="rs_yi", tag="rs_yi")
    nc.vector.tensor_scalar(
        yi[:], m[:].bitcast(I32), 1, None, op0=OP.arith_shift_right
    )
    nc.vector.tensor_scalar(
        yi[:], yi[:], -1, 0x5F3759DF, op0=OP.mult, op1=OP.add
    )
    y = yi[:].bitcast(F32)
    half = pool.tile(shp, F32, name="rs_half", tag="rs_half")
    nc.vector.tensor_scalar_mul(half[:], m[:], 0.5)
    t1 = pool.tile(shp, F32, name="rs_t1", tag="rs_t1")
    for it in range(3):
        nc.vector.tensor_tensor(t1[:], y, y, op=OP.mult)
        nc.vector.tensor_tensor(t1[:], t1[:], half[:], op=OP.mult)
        nc.vector.tensor_scalar(t1[:], t1[:], -1.0, 1.5, op0=OP.mult, op1=OP.add)
        if it < 2:
            nc.vector.tensor_tensor(y, y, t1[:], op=OP.mult)
        else:
            nc.vector.tensor_tensor(out, y, t1[:], op=OP.mult)
    return out


def build_full():
    nc = bacc.Bacc(None, target_bir_lowering=False, num_devices=NC)

    x_in = nc.dram_tensor("x_in", [NTOK, DIM], F32, kind="ExternalInput")
    Wq6 = nc.dram_tensor("Wq6", [6, 768, 768], F32R, kind="ExternalInput")
    Wk6 = nc.dram_tensor("Wk6", [6, 768, 768], F32R, kind="ExternalInput")
    Wv6 = nc.dram_tensor("Wv6", [6, 768, 768], F32R, kind="ExternalInput")
    Wo6 = nc.dram_tensor("Wo6", [6, 768, 768], F32R, kind="ExternalInput")
    Wmg6 = nc.dram_tensor("Wmg6", [6, 768, 24], F32R, kind="ExternalInput")
    kg6 = nc.dram_tensor("kg6", [6, 768], F32, kind="ExternalInput")
    Win6 = nc.dram_tensor("Win6", [6, 768, 4096], F32R, kind="ExternalInput")
    Wout6 = nc.dram_tensor("Wout6", [6, 2048, 768], F32R, kind="ExternalInput")
    WqT = nc.dram_tensor("WqT", [2, 768, 768], F32R, kind="ExternalInput")
    WkT = nc.dram_tensor("WkT", [2, 768, 768], F32R, kind="ExternalInput")
    WvT = nc.dram_tensor("WvT", [2, 768, 768], F32R, kind="ExternalInput")
    WoT = nc.dram_tensor("WoT", [2, 768, 768], F32R, kind="ExternalInput")
    WmgT = nc.dram_tensor("WmgT", [2, 768, 24], F32R, kind="ExternalInput")
    kgT = nc.dram_tensor("kgT", [2, 768], F32, kind="ExternalInput")
    WinT = nc.dram_tensor("WinT", [2, 768, 4096], F32R, kind="ExternalInput")
    WoutT = nc.dram_tensor("WoutT", [2, 2048, 768], F32R, kind="ExternalInput")
    vrW = nc.dram_tensor("vrW", [768, 768], F32R, kind="ExternalInput")
    rotc_i = nc.dram_tensor("rotc", [128, 256], F32, kind="ExternalInput")
    rots_i = nc.dram_tensor("rots", [128, 256], F32, kind="ExternalInput")
    pmat_i = nc.dram_tensor("pmat", [128, 128], F32, kind="ExternalInput")
    mask_i = nc.dram_tensor("maskf", [128, 128], F32, kind="ExternalInput")
    x_out = nc.dram_tensor("x_out", [NTOK, DIM], F32, kind="ExternalOutput")

    with tile.TileContext(nc) as tc:
        with ExitStack() as top:
            const = top.enter_context(tc.tile_pool(name="const", bufs=1))
            dram = top.enter_context(
                tc.tile_pool(name="dramp", bufs=1, space="DRAM")
            )
            xpool = top.enter_context(tc.tile_pool(name="xpool", bufs=1))

            x_sb = xpool.tile([128, NT, 768], F32, name="x_sb")
            nc.sync.dma_start(
                x_sb[:], x_in[:].rearrange("(t p) d -> p t d", p=128)
            )

            ident_f = const.tile([128, 128], F32, name="ident_f")
            make_identity(nc, ident_f)
            ident = const.tile([128, 128], F32R, name="ident")
            nc.vector.tensor_copy(ident[:], ident_f[:])

            ld_f = const.tile([128, 128], F32, name="ld_f")
            nc.sync.dma_start(ld_f[:], pmat_i[:])
            pmat = const.tile([128, 128], F32R, name="pmat_t")
            nc.vector.tensor_copy(pmat[:], ld_f[:])
            mk_f = const.tile([128, 128], F32, name="mk_f")
            nc.sync.dma_start(mk_f[:], mask_i[:])
            mask_b = const.tile([128, 128], BF16, name="mask_b")
            nc.vector.tensor_copy(mask_b[:], mk_f[:])
            rotc = const.tile([128, 256], F32, name="rotc_t")
            nc.sync.dma_start(rotc[:], rotc_i[:])
            rots = const.tile([128, 256], F32, name="rots_t")
            nc.sync.dma_start(rots[:], rots_i[:])

            # DRAM bounce buffers (chunk layout sll,slh,tl,b,d)
            rv_t_d = dram.tile([NTOK, 768], F32, name="rv_t_d")
            rv_ain = dram.tile([NC, 4, 8, 4, 2, 768], F32, name="rv_ain")
            rv_s_d = dram.tile([NC, 4, 8, 4, 2, 768], F32, name="rv_s_d")
            xa_in = dram.tile([NC, 4, 8, 4, 2, 768], F32, name="xa_in")
            xa_out = dram.tile([NC, 4, 8, 4, 2, 768], F32, name="xa_out")

            # rv in s-domain: per-group view, dims (slh | sll c tl | b | d)
            rv_s_view = rv_s_d[:].rearrange(
                "c sll slh tl b d -> slh sll c tl b d"
            )

            # ---- pre: rv projection + rv all-to-all ------------------------
            _pre_rv(nc, tc, x_sb, ident, vrW, rv_t_d, rv_ain, rv_s_d)

            # ---- layers ----------------------------------------------------
            for li in range(3):
                _attn_layer(nc, tc, li, x_sb, ident, rv_t_d, Wq6, Wk6, Wv6,
                            Wo6, Wmg6, kg6)
                _ff_layer(nc, tc, li, x_sb, ident, Win6, Wout6, "s")

            _a2a_t2s(nc, tc, x_sb, xa_in, xa_out)

            _time_layer(nc, tc, 0, x_sb, ident, pmat, rotc, rots, mask_b,
                        rv_s_view, WqT, WkT, WvT, WoT, WmgT, kgT)
            _ff_layer(nc, tc, 0, x_sb, ident, WinT, WoutT, "t")

            _a2a_s2t(nc, tc, x_sb, xa_in, xa_out)

            for li in range(3, 6):
                _attn_layer(nc, tc, li, x_sb, ident, rv_t_d, Wq6, Wk6, Wv6,
                            Wo6, Wmg6, kg6)
                _ff_layer(nc, tc, li, x_sb, ident, Win6, Wout6, "s")

            _a2a_t2s(nc, tc, x_sb, xa_in, xa_out)

            _time_layer(nc, tc, 1, x_sb, ident, pmat, rotc, rots, mask_b,
                        rv_s_view, WqT, WkT, WvT, WoT, WmgT, kgT)
            _ff_layer(nc, tc, 1, x_sb, ident, WinT, WoutT, "t")

            _final_norm(nc, tc, x_sb, x_out)

    nc.compile()

    in_names = []
    out_names = []
    out_avals = []

    pname = nc.partition_id_tensor.name if nc.partition_id_tensor else None
    for alloc in nc.m.functions[0].allocations:
        if not isinstance(alloc, mybir.MemoryLocationSet):
            continue
        if not alloc.memorylocations:
            continue
        name = alloc.memorylocations[0].name
        if alloc.kind == "ExternalInput" and name != pname:
            in_names.append(name)
        elif alloc.kind == "ExternalOutput":
            out_names.append(name)
            out_avals.append(
                jax.core.ShapedArray(
                    tuple(alloc.tensor_shape), mybir.dt.np(alloc.dtype)
                )
            )
    return nc, in_names, out_names, out_avals


# ---------------------------------------------------------------------------
# all-to-all helpers.  Bounce buffers are [NC, 4(sll), 8(slh), 4(tl), 2(b), d]
# (chunk layout sll,slh,tl,b,d).  t-domain sbuf: tile=(th,tl,b), p=(jh,sll,slh)
# with s = 128*th + 32*jh + 4*slh + sll.  s-domain sbuf: tile=(slh,b),
# p=(sll,c,tl) with t = 4*c + tl, sl = 4*slh + sll.
# ---------------------------------------------------------------------------
def _send_t2s(nc, src_sb, xa_in):
    """t-domain SBUF -> bounce chunks (8 DMAs)."""
    xv = src_sb[:].rearrange("p (th tl b) d -> p th tl b d", th=2, tl=4)
    for th in range(2):
        for jh in range(4):
            j = th * 4 + jh
            nc.sync.dma_start(
                xa_in[ds(j, 1), :, :, :, :, :],
                xv[jh * 32 : (jh + 1) * 32, ds(th, 1), :, :, :],
            )


def _a2a_t2s(nc, tc, x_sb, xa_in, xa_out):
    _send_t2s(nc, x_sb, xa_in)
    nc.gpsimd.collective_compute(
        "AllToAll", OP.bypass, replica_groups=RG,
        ins=[xa_in[:].opt()], outs=[xa_out[:].opt()],
    )
    # chunks (c) -> s-domain sbuf (32 DMAs)
    xo = xa_out[:].rearrange("c sll slh tl b d -> c sll tl slh b d")
    for sll in range(4):
        for c in range(NC):
            nc.sync.dma_start(
                x_sb[sll * 32 + c * 4 : sll * 32 + c * 4 + 4, :, :],
                xo[ds(c, 1), ds(sll, 1), :, :, :, :],
            )


def _a2a_s2t(nc, tc, x_sb, xa_in, xa_out):
    # s-domain sbuf -> bounce chunks (32 DMAs)
    xi = xa_in[:].rearrange("j sll slh tl b d -> j sll tl slh b d")
    for j in range(NC):
        for sll in range(4):
            nc.sync.dma_start(
                xi[ds(j, 1), ds(sll, 1), :, :, :, :],
                x_sb[sll * 32 + j * 4 : sll * 32 + j * 4 + 4, :, :],
            )
    nc.gpsimd.collective_compute(
        "AllToAll", OP.bypass, replica_groups=RG,
        ins=[xa_in[:].opt()], outs=[xa_out[:].opt()],
    )
    # chunks (cs = th*4+jh) -> t-domain sbuf (8 DMAs)
    xv = x_sb[:].rearrange("p (th tl b) d -> p th tl b d", th=2, tl=4)
    for cs in range(NC):
        th, jh = cs // 4, cs % 4
        nc.sync.dma_start(
            xv[jh * 32 : (jh + 1) * 32, ds(th, 1), :, :, :],
            xa_out[ds(cs, 1), :, :, :, :, :],
        )


# ---------------------------------------------------------------------------
# pre: rv = rmsnorm(tokens) @ vrW  (t-domain) + AllToAll to s-domain
# ---------------------------------------------------------------------------
def _pre_rv(nc, tc, x_sb, ident, vrW, rv_t_d, rv_ain, rv_s_d):
    with ExitStack() as ctx:
        wp = ctx.enter_context(tc.tile_pool(name="vrw", bufs=1))
        vw = wp.tile([128, KT, 768], F32R, name="vw")
        nc.sync.dma_start(vw[:], vrW[:].rearrange("(kt p) m -> p kt m", p=128))

        rvp = ctx.enter_context(tc.tile_pool(name="rvp", bufs=1))
        rv_sb = rvp.tile([128, NT, 768], F32, name="rv_sb")
        sp = ctx.enter_context(tc.tile_pool(name="prsp", bufs=1))
        np_ = ctx.enter_context(tc.tile_pool(name="prnp", bufs=2))
        ps_tr = ctx.enter_context(
            tc.tile_pool(name="prps_tr", bufs=2, space="PSUM")
        )
        ps_pj = ctx.enter_context(
            tc.tile_pool(name="prps_pj", bufs=2, space="PSUM")
        )

        for sv in range(8):
            sq = sp.tile([128, 768], F32, name="prsq", tag="prsq")
            ss = np_.tile([128, 2], F32, name="prss", tag="prss")
            for j in range(2):
                nc.scalar.activation(
                    sq[:], x_sb[:, ds(sv + 8 * j, 1), :].squeeze(1), AF.Square,
                    accum_out=ss[:, j : j + 1],
                )
            inv = np_.tile([128, 2], F32, name="prinv", tag="prinv")
            _emit_rsqrt(nc, np_, inv[:], ss[:], 1.0 / 768.0, 1e-6, 1e-30)
            tn_t = sp.tile([128, 2, 768], F32R, name="prtn", tag="prtn")
            for j in range(2):
                nc.vector.tensor_scalar_mul(
                    tn_t[:, j, :], x_sb[:, ds(sv + 8 * j, 1), :].squeeze(1),
                    inv[:, j : j + 1],
                )
            tn_f = sp.tile([128, KT, 256], F32R, name="prtf", tag="prtf")
            for kt in range(KT):
                pt = ps_tr.tile([128, 256], F32R, name="prpt", tag="prps_tr")
                for j in range(2):
                    nc.tensor.transpose(
                        pt[:, j * 128 : (j + 1) * 128],
                        tn_t[:, j, kt * 128 : (kt + 1) * 128],
                        ident[:],
                    )
                nc.scalar.copy(tn_f[:, kt, :], pt[:].bitcast(F32))
            for j in range(2):
                for nh in range(2):
                    pv = ps_pj.tile([128, 384], F32, name="prpv", tag="prps_pj")
                    for kt in range(KT):
                        nc.tensor.matmul(
                            pv[:],
                            lhsT=tn_f[:, kt, j * 128 : (j + 1) * 128],
                            rhs=vw[:, kt, nh * 384 : (nh + 1) * 384],
                            start=(kt == 0),
                            stop=(kt == KT - 1),
                        )
                    nc.scalar.copy(
                        rv_sb[:, sv + 8 * j, nh * 384 : (nh + 1) * 384], pv[:]
                    )

        nc.sync.dma_start(
            rv_t_d[:].rearrange("(t p) d -> p t d", p=128), rv_sb[:]
        )
        _send_t2s(nc, rv_sb, rv_ain)
        nc.gpsimd.collective_compute(
            "AllToAll", OP.bypass, replica_groups=RG,
            ins=[rv_ain[:].opt()], outs=[rv_s_d[:].opt()],
        )


# ---------------------------------------------------------------------------
# space attention layer (t-domain; 8 seqs of 256 tokens)
# ---------------------------------------------------------------------------
def _attn_layer(nc, tc, L, x_sb, ident, rv_in, Wq3, Wk3, Wv3, Wo3, Wmg3, kg3):
    with ExitStack() as ctx:
        wp = ctx.enter_context(tc.tile_pool(name=f"wq{L}", bufs=1))
        wq = wp.tile([128, KT, 768], F32R, name=f"wq_t{L}")
        wk = wp.tile([128, KT, 768], F32R, name=f"wk_t{L}")
        wv = wp.tile([128, KT, 768], F32R, name=f"wv_t{L}")
        wo = wp.tile([128, KT, 768], F32R, name=f"wo_t{L}")
        wmg = wp.tile([128, KT, 24], F32R, name=f"wmg_t{L}")
        kgbc = wp.tile([128, 768], F32, name=f"kgbc{L}")
        for w_t, W in ((wq, Wq3), (wk, Wk3), (wv, Wv3), (wo, Wo3), (wmg, Wmg3)):
            nc.sync.dma_start(
                w_t[:], W[L].rearrange("(kt p) m -> p kt m", p=128)
            )
        nc.sync.dma_start(kgbc[:], kg3[L : L + 1, :].partition_broadcast(128))

        sp = ctx.enter_context(tc.tile_pool(name=f"sp{L}", bufs=1))
        sp2 = ctx.enter_context(tc.tile_pool(name=f"sp2{L}", bufs=2))
        hp = ctx.enter_context(tc.tile_pool(name=f"hp{L}", bufs=3))
        np_ = ctx.enter_context(tc.tile_pool(name=f"np{L}", bufs=2))
        ps_tr = ctx.enter_context(
            tc.tile_pool(name=f"ps_tr{L}", bufs=2, space="PSUM")
        )
        ps_pj = ctx.enter_context(
            tc.tile_pool(name=f"ps_pj{L}", bufs=2, space="PSUM")
        )
        ps_S = ctx.enter_context(
            tc.tile_pool(name=f"ps_S{L}", bufs=2, space="PSUM")
        )
        ps_O = ctx.enter_context(
            tc.tile_pool(name=f"ps_O{L}", bufs=2, space="PSUM")
        )

        rv_tv = rv_in[:].rearrange("(th r p) d -> p th r d", th=2, p=128)

        def seq_body(sv):
            # ---- rv slice for this seq (tiles sv, sv+8)
            rv_sl = sp.tile([128, 2, 768], F32, name="rv_sl", tag="rv_sl")
            nc.sync.dma_start(rv_sl[:], rv_tv[:, :, ds(sv, 1), :])
            # ---- rmsnorm
            sq = sp.tile([128, 768], F32, name="sq", tag="sq")
            ss = np_.tile([128, 2], F32, name="ss", tag="ss")
            for j in range(2):
                nc.scalar.activation(
                    sq[:], x_sb[:, ds(sv + 8 * j, 1), :].squeeze(1), AF.Square,
                    accum_out=ss[:, j : j + 1],
                )
            inv = np_.tile([128, 2], F32, name="inv", tag="inv")
            _emit_rsqrt(nc, np_, inv[:], ss[:], 1.0 / 768.0, 1e-6, 1e-30)
            tn_t = sp.tile([128, 2, 768], F32R, name="tn_t", tag="tn_t")
            for j in range(2):
                nc.vector.tensor_scalar_mul(
                    tn_t[:, j, :], x_sb[:, ds(sv + 8 * j, 1), :].squeeze(1),
                    inv[:, j : j + 1],
                )
            # ---- transpose tn -> tn_f
            tn_f = sp.tile([128, KT, 256], F32R, name="tn_f", tag="tn_f")
            for kt in range(KT):
                pt = ps_tr.tile([128, 256], F32R, name="pt_tn", tag="ps_tr")
                for j in range(2):
                    nc.tensor.transpose(
                        pt[:, j * 128 : (j + 1) * 128],
                        tn_t[:, j, kt * 128 : (kt + 1) * 128],
                        ident[:],
                    )
                nc.scalar.copy(tn_f[:, kt, :], pt[:].bitcast(F32))
            # ---- q projection (feature-major)
            q_f = sp2.tile([128, KT, 256], F32R, name="q_f", tag="q_f")
            for m in range(KT):
                pq = ps_pj.tile([128, 384], F32, name="pq", tag="ps_pj")
                for kt in range(KT):
                    nc.tensor.matmul(
                        pq[:, :256],
                        lhsT=wq[:, kt, m * 128 : (m + 1) * 128],
                        rhs=tn_f[:, kt, :],
                        start=(kt == 0),
                        stop=(kt == KT - 1),
                    )
                nc.scalar.copy(q_f[:, m, :], pq[:, :256])
            # ---- k projection (token-major) + l2norm * kgamma
            kraw = sp.tile([128, 2, 768], F32R, name="kraw", tag="kraw")
            for j in range(2):
                for nh in range(2):
                    pk = ps_pj.tile([128, 384], F32, name="pk", tag="ps_pj")
                    for kt in range(KT):
                        nc.tensor.matmul(
                            pk[:],
                            lhsT=tn_f[:, kt, j * 128 : (j + 1) * 128],
                            rhs=wk[:, kt, nh * 384 : (nh + 1) * 384],
                            start=(kt == 0),
                            stop=(kt == KT - 1),
                        )
                    nc.scalar.copy(kraw[:, j, nh * 384 : (nh + 1) * 384], pk[:])
            kss = np_.tile([128, 24], F32, name="kss", tag="kss")
            for j in range(2):
                nc.vector.tensor_tensor(
                    sq[:], kraw[:, j, :].bitcast(F32),
                    kraw[:, j, :].bitcast(F32), op=OP.mult
                )
                nc.vector.tensor_reduce(
                    out=kss[:, j * 12 : (j + 1) * 12],
                    in_=sq[:].rearrange("p (h d) -> p h d", h=H),
                    axis=mybir.AxisListType.X,
                    op=OP.add,
                )
            kinv = np_.tile([128, 24], F32, name="kinv", tag="kinv")
            _emit_rsqrt(nc, np_, kinv[:], kss[:], 1.0, 0.0, 1e-24)
            kib = sp.tile([128, 768], F32, name="kib", tag="kib")
            for j in range(2):
                nc.vector.tensor_copy(
                    kib[:].rearrange("p (h d) -> p h d", h=H),
                    kinv[:, j * 12 : (j + 1) * 12]
                    .unsqueeze(2)
                    .broadcast_to([128, H, DH]),
                )
                nc.vector.tensor_tensor(kib[:], kib[:], kgbc[:], op=OP.mult)
                nc.vector.tensor_tensor(
                    kraw[:, j, :], kraw[:, j, :].bitcast(F32), kib[:],
                    op=OP.mult,
                )
            k_f = sp2.tile([128, KT, 256], F32R, name="k_f", tag="k_f")
            for kt in range(KT):
                pt = ps_tr.tile([128, 256], F32R, name="pt_k", tag="ps_tr")
                for j in range(2):
                    nc.tensor.transpose(
                        pt[:, j * 128 : (j + 1) * 128],
                        kraw[:, j, kt * 128 : (kt + 1) * 128],
                        ident[:],
                    )
                nc.scalar.copy(k_f[:, kt, :], pt[:].bitcast(F32))
            # ---- mix / gates (sigmoid via tanh)
            mgs = np_.tile([128, 2, 24], F32, name="mgs", tag="mgs")
            for j in range(2):
                pm = ps_O.tile([128, 65], F32, name="pm", tag="ps_O")
                for kt in range(KT):
                    nc.tensor.matmul(
                        pm[:, :24],
                        lhsT=tn_f[:, kt, j * 128 : (j + 1) * 128],
                        rhs=wmg[:, kt, :],
                        start=(kt == 0),
                        stop=(kt == KT - 1),
                    )
                nc.scalar.activation(mgs[:, j, :], pm[:, :24], AF.Tanh, scale=0.5)
            nc.vector.tensor_scalar(
                mgs[:], mgs[:], 0.5, 0.5, op0=OP.mult, op1=OP.add
            )
            # ---- v projection + value-residual lerp -> v1 (bf16, |1 col)
            v1 = sp2.tile([128, 2, H, 65], BF16, name="v1", tag="v1")
            mixb = kib
            tdt = sq[:, 0:384]
            for j in range(2):
                nc.vector.tensor_copy(
                    mixb[:].rearrange("p (h d) -> p h d", h=H),
                    mgs[:, j, 0:12].unsqueeze(2).broadcast_to([128, H, DH]),
                )
                for nh in range(2):
                    pv = ps_pj.tile([128, 384], F32, name="pv", tag="ps_pj")
                    for kt in range(KT):
                        nc.tensor.matmul(
                            pv[:],
                            lhsT=tn_f[:, kt, j * 128 : (j + 1) * 128],
                            rhs=wv[:, kt, nh * 384 : (nh + 1) * 384],
                            start=(kt == 0),
                            stop=(kt == KT - 1),
                        )
                    nc.vector.tensor_tensor(
                        tdt, rv_sl[:, j, nh * 384 : (nh + 1) * 384], pv[:],
                        op=OP.subtract,
                    )
                    nc.vector.tensor_tensor(
                        tdt, tdt, mixb[:, nh * 384 : (nh + 1) * 384],
                        op=OP.mult,
                    )
                    nc.vector.tensor_tensor(
                        v1[:, j, 6 * nh : 6 * nh + 6, 0:64],
                        pv[:].rearrange("p (h d) -> p h d", h=6),
                        tdt.rearrange("p (h d) -> p h d", h=6),
                        op=OP.add,
                    )
                nc.vector.memset(v1[:, j, :, 64:65], 1.0)
            # ---- attention per head: scores k-major (no transposes)
            o_t = tn_t
            for h in range(H):
                pt_b = hp.tile([128, 2, 256], BF16, name="pt_b", tag="pt_b")
                st = hp.tile([128, 256], F32, name="st", tag="st")
                rec = np_.tile([128, 1], F32, name="rec", tag="rec")
                mt, po = h // 2, 64 * (h % 2)
                for kvt in range(2):
                    pS = ps_S.tile([128, 256], F32, name="pS", tag="ps_S")
                    nc.tensor.matmul(
                        pS[:],
                        lhsT=k_f[po : po + 64, mt, kvt * 128 : (kvt + 1) * 128],
                        rhs=q_f[po : po + 64, mt, :],
                        start=True,
                        stop=True,
                    )
                    nc.scalar.activation(st[:], pS[:], AF.Tanh)
                    nc.scalar.activation(pt_b[:, kvt, :], st[:], AF.Exp, scale=50.0)
                for qt in range(2):
                    pO = ps_O.tile([128, 65], F32, name="pO", tag="ps_O")
                    for kvt in range(2):
                        nc.tensor.matmul(
                            pO[:],
                            lhsT=pt_b[:, kvt, qt * 128 : (qt + 1) * 128],
                            rhs=v1[:, kvt, h, :],
                            start=(kvt == 0),
                            stop=(kvt == 1),
                        )
                    nc.vector.reciprocal(rec[:], pO[:, 64:65])
                    nc.vector.tensor_tensor(
                        rec[:], rec[:], mgs[:, qt, 12 + h : 13 + h], op=OP.mult
                    )
                    nc.vector.tensor_scalar_mul(
                        o_t[:, qt, 64 * h : 64 * h + 64], pO[:, 0:64], rec[:]
                    )
            # ---- transpose o -> o_f, then Wo and residual add
            o_f = tn_f
            for kt in range(KT):
                pt = ps_tr.tile([128, 256], F32R, name="pt_o", tag="ps_tr")
                for j in range(2):
                    nc.tensor.transpose(
                        pt[:, j * 128 : (j + 1) * 128],
                        o_t[:, j, kt * 128 : (kt + 1) * 128],
                        ident[:],
                    )
                nc.scalar.copy(o_f[:, kt, :], pt[:].bitcast(F32))
            for j in range(2):
                for nh in range(2):
                    px = ps_pj.tile([128, 384], F32, name="px", tag="ps_pj")
                    for kt in range(KT):
                        nc.tensor.matmul(
                            px[:],
                            lhsT=o_f[:, kt, j * 128 : (j + 1) * 128],
                            rhs=wo[:, kt, nh * 384 : (nh + 1) * 384],
                            start=(kt == 0),
                            stop=(kt == KT - 1),
                        )
                    xs = x_sb[:, ds(sv + 8 * j, 1), nh * 384 : (nh + 1) * 384]
                    xs = xs.squeeze(1)
                    nc.vector.tensor_tensor(xs, xs, px[:], op=OP.add)

        for _sv in range(8):
            seq_body(_sv)


# ---------------------------------------------------------------------------
# time attention layer (s-domain; 8 groups of 2 tiles; 4 causal seqs of 32
# per 128-token tile, rotary + block-diag causal mask)
# ---------------------------------------------------------------------------
def _time_layer(nc, tc, L, x_sb, ident, pmat, rotc, rots, mask_b, rv_view,
                Wq2, Wk2, Wv2, Wo2, Wmg2, kg2):
    with ExitStack() as ctx:
        wp = ctx.enter_context(tc.tile_pool(name=f"twq{L}", bufs=1))
        wq = wp.tile([128, KT, 768], F32R, name=f"twq_t{L}")
        wk = wp.tile([128, KT, 768], F32R, name=f"twk_t{L}")
        wv = wp.tile([128, KT, 768], F32R, name=f"twv_t{L}")
        wo = wp.tile([128, KT, 768], F32R, name=f"two_t{L}")
        wmg = wp.tile([128, KT, 24], F32R, name=f"twmg_t{L}")
        kgbc = wp.tile([128, 768], F32, name=f"tkgbc{L}")
        for w_t, W in ((wq, Wq2), (wk, Wk2), (wv, Wv2), (wo, Wo2), (wmg, Wmg2)):
            nc.sync.dma_start(
                w_t[:], W[L].rearrange("(kt p) m -> p kt m", p=128)
            )
        nc.sync.dma_start(kgbc[:], kg2[L : L + 1, :].partition_broadcast(128))

        sp = ctx.enter_context(tc.tile_pool(name=f"tsp{L}", bufs=1))
        sp2 = ctx.enter_context(tc.tile_pool(name=f"tsp2{L}", bufs=2))
        hp = ctx.enter_context(tc.tile_pool(name=f"thp{L}", bufs=3))
        np_ = ctx.enter_context(tc.tile_pool(name=f"tnp{L}", bufs=2))
        ps_tr = ctx.enter_context(
            tc.tile_pool(name=f"tps_tr{L}", bufs=2, space="PSUM")
        )
        ps_pj = ctx.enter_context(
            tc.tile_pool(name=f"tps_pj{L}", bufs=2, space="PSUM")
        )
        ps_S = ctx.enter_context(
            tc.tile_pool(name=f"tps_S{L}", bufs=2, space="PSUM")
        )
        ps_O = ctx.enter_context(
            tc.tile_pool(name=f"tps_O{L}", bufs=2, space="PSUM")
        )

        def grp_body(gv):
            off = gv * 2
            rv_sl = sp.tile([128, 2, 768], F32, name="trv", tag="trv")
            for sll in range(4):
                nc.sync.dma_start(
                    rv_sl[sll * 32 : (sll + 1) * 32, :, :],
                    rv_view[ds(gv, 1), ds(sll, 1), :, :, :, :],
                )
            # ---- rmsnorm
            sq = sp.tile([128, 768], F32, name="tsq", tag="tsq")
            ss = np_.tile([128, 2], F32, name="tss", tag="tss")
            for j in range(2):
                nc.scalar.activation(
                    sq[:], x_sb[:, ds(off + j, 1), :].squeeze(1), AF.Square,
                    accum_out=ss[:, j : j + 1],
                )
            inv = np_.tile([128, 2], F32, name="tinv", tag="tinv")
            _emit_rsqrt(nc, np_, inv[:], ss[:], 1.0 / 768.0, 1e-6, 1e-30)
            tn_t = sp.tile([128, 2, 768], F32R, name="ttn", tag="ttn")
            for j in range(2):
                nc.vector.tensor_scalar_mul(
                    tn_t[:, j, :], x_sb[:, ds(off + j, 1), :].squeeze(1),
                    inv[:, j : j + 1],
                )
            # ---- transpose tn -> tn_f
            tn_f = sp.tile([128, KT, 256], F32R, name="ttf", tag="ttf")
            for kt in range(KT):
                pt = ps_tr.tile([128, 256], F32R, name="tpt", tag="tps_tr")
                for j in range(2):
                    nc.tensor.transpose(
                        pt[:, j * 128 : (j + 1) * 128],
                        tn_t[:, j, kt * 128 : (kt + 1) * 128],
                        ident[:],
                    )
                nc.scalar.copy(tn_f[:, kt, :], pt[:].bitcast(F32))
            # ---- q projection (feature-major) + rotary -> q_r (bf16)
            q_f = sp2.tile([128, KT, 256], F32R, name="tq_f", tag="tq_f")
            for m in range(KT):
                pq = ps_pj.tile([128, 384], F32, name="tpq", tag="tps_pj")
                for kt in range(KT):
                    nc.tensor.matmul(
                        pq[:, :256],
                        lhsT=wq[:, kt, m * 128 : (m + 1) * 128],
                        rhs=tn_f[:, kt, :],
                        start=(kt == 0),
                        stop=(kt == KT - 1),
                    )
                nc.scalar.copy(q_f[:, m, :], pq[:, :256])
            q_r = sp2.tile([128, KT, 256], BF16, name="tq_r", tag="tq_r")
            t1 = sp.tile([128, 256], F32, name="trt1", tag="trt1")
            t2 = sp.tile([128, 256], F32, name="trt2", tag="trt2")
            for m in range(KT):
                pr = ps_tr.tile([128, 256], F32, name="tpr", tag="tps_tr")
                nc.tensor.matmul(
                    pr[:], lhsT=pmat[:], rhs=q_f[:, m, :],
                    start=True, stop=True,
                )
                nc.vector.tensor_tensor(
                    t1[:], q_f[:, m, :].bitcast(F32), rotc[:], op=OP.mult
                )
                nc.vector.tensor_tensor(t2[:], pr[:], rots[:], op=OP.mult)
                nc.vector.tensor_tensor(q_r[:, m, :], t1[:], t2[:], op=OP.add)
            # ---- k projection (token-major) + l2norm * kgamma
            kraw = sp.tile([128, 2, 768], F32R, name="tkraw", tag="tkraw")
            for j in range(2):
                for nh in range(2):
                    pk = ps_pj.tile([128, 384], F32, name="tpk", tag="tps_pj")
                    for kt in range(KT):
                        nc.tensor.matmul(
                            pk[:],
                            lhsT=tn_f[:, kt, j * 128 : (j + 1) * 128],
                            rhs=wk[:, kt, nh * 384 : (nh + 1) * 384],
                            start=(kt == 0),
                            stop=(kt == KT - 1),
                        )
                    nc.scalar.copy(kraw[:, j, nh * 384 : (nh + 1) * 384], pk[:])
            kss = np_.tile([128, 24], F32, name="tkss", tag="tkss")
            for j in range(2):
                nc.vector.tensor_tensor(
                    sq[:], kraw[:, j, :].bitcast(F32),
                    kraw[:, j, :].bitcast(F32), op=OP.mult
                )
                nc.vector.tensor_reduce(
                    out=kss[:, j * 12 : (j + 1) * 12],
                    in_=sq[:].rearrange("p (h d) -> p h d", h=H),
                    axis=mybir.AxisListType.X,
                    op=OP.add,
                )
            kinv = np_.tile([128, 24], F32, name="tkinv", tag="tkinv")
            _emit_rsqrt(nc, np_, kinv[:], kss[:], 1.0, 0.0, 1e-24)
            kib = sp.tile([128, 768], F32, name="tkib", tag="tkib")
            for j in range(2):
                nc.vector.tensor_copy(
                    kib[:].rearrange("p (h d) -> p h d", h=H),
                    kinv[:, j * 12 : (j + 1) * 12]
                    .unsqueeze(2)
                    .broadcast_to([128, H, DH]),
                )
                nc.vector.tensor_tensor(kib[:], kib[:], kgbc[:], op=OP.mult)
                nc.vector.tensor_tensor(
                    kraw[:, j, :], kraw[:, j, :].bitcast(F32), kib[:],
                    op=OP.mult,
                )
            # ---- transpose k -> k_f + rotary -> k_r (bf16)
            k_f = sp2.tile([128, KT, 256], F32R, name="tk_f", tag="tk_f")
            for kt in range(KT):
                pt = ps_tr.tile([128, 256], F32R, name="tptk", tag="tps_tr")
                for j in range(2):
                    nc.tensor.transpose(
                        pt[:, j * 128 : (j + 1) * 128],
                        kraw[:, j, kt * 128 : (kt + 1) * 128],
                        ident[:],
                    )
                nc.scalar.copy(k_f[:, kt, :], pt[:].bitcast(F32))
            k_r = sp2.tile([128, KT, 256], BF16, name="tk_r", tag="tk_r")
            for m in range(KT):
                pr = ps_tr.tile([128, 256], F32, name="tprk", tag="tps_tr")
                nc.tensor.matmul(
                    pr[:], lhsT=pmat[:], rhs=k_f[:, m, :],
                    start=True, stop=True,
                )
                nc.vector.tensor_tensor(
                    t1[:], k_f[:, m, :].bitcast(F32), rotc[:], op=OP.mult
                )
                nc.vector.tensor_tensor(t2[:], pr[:], rots[:], op=OP.mult)
                nc.vector.tensor_tensor(k_r[:, m, :], t1[:], t2[:], op=OP.add)
            # ---- mix / gates
            mgs = np_.tile([128, 2, 24], F32, name="tmgs", tag="tmgs")
            for j in range(2):
                pm = ps_O.tile([128, 65], F32, name="tpm", tag="tps_O")
                for kt in range(KT):
                    nc.tensor.matmul(
                        pm[:, :24],
                        lhsT=tn_f[:, kt, j * 128 : (j + 1) * 128],
                        rhs=wmg[:, kt, :],
                        start=(kt == 0),
                        stop=(kt == KT - 1),
                    )
                nc.scalar.activation(mgs[:, j, :], pm[:, :24], AF.Tanh, scale=0.5)
            nc.vector.tensor_scalar(
                mgs[:], mgs[:], 0.5, 0.5, op0=OP.mult, op1=OP.add
            )
            # ---- v projection + value-residual lerp -> v1
            v1 = sp2.tile([128, 2, H, 65], BF16, name="tv1", tag="tv1")
            mixb = kib
            tdt = sq[:, 0:384]
            for j in range(2):
                nc.vector.tensor_copy(
                    mixb[:].rearrange("p (h d) -> p h d", h=H),
                    mgs[:, j, 0:12].unsqueeze(2).broadcast_to([128, H, DH]),
                )
                for nh in range(2):
                    pv = ps_pj.tile([128, 384], F32, name="tpv", tag="tps_pj")
                    for kt in range(KT):
                        nc.tensor.matmul(
                            pv[:],
                            lhsT=tn_f[:, kt, j * 128 : (j + 1) * 128],
                            rhs=wv[:, kt, nh * 384 : (nh + 1) * 384],
                            start=(kt == 0),
                            stop=(kt == KT - 1),
                        )
                    nc.vector.tensor_tensor(
                        tdt, rv_sl[:, j, nh * 384 : (nh + 1) * 384], pv[:],
                        op=OP.subtract,
                    )
                    nc.vector.tensor_tensor(
                        tdt, tdt, mixb[:, nh * 384 : (nh + 1) * 384],
                        op=OP.mult,
                    )
                    nc.vector.tensor_tensor(
                        v1[:, j, 6 * nh : 6 * nh + 6, 0:64],
                        pv[:].rearrange("p (h d) -> p h d", h=6),
                        tdt.rearrange("p (h d) -> p h d", h=6),
                        op=OP.add,
                    )
                nc.vector.memset(v1[:, j, :, 64:65], 1.0)
            # ---- attention per (tile, head): causal via mask multiply
            o_t = tn_t
            for j in range(2):
                for h in range(H):
                    mt, po = h // 2, 64 * (h % 2)
                    pS = ps_S.tile([128, 128], F32, name="tpS", tag="tps_S")
                    nc.tensor.matmul(
                        pS[:],
                        lhsT=k_r[po : po + 64, mt, j * 128 : (j + 1) * 128],
                        rhs=q_r[po : po + 64, mt, j * 128 : (j + 1) * 128],
                        start=True,
                        stop=True,
                    )
                    st = hp.tile([128, 128], F32, name="tst", tag="tst")
                    nc.scalar.activation(st[:], pS[:], AF.Tanh)
                    eb = hp.tile([128, 128], BF16, name="teb", tag="teb")
                    nc.scalar.activation(eb[:], st[:], AF.Exp, scale=50.0)
                    me = hp.tile([128, 128], BF16, name="tme", tag="tme")
                    nc.vector.tensor_tensor(me[:], eb[:], mask_b[:], op=OP.mult)
                    pO = ps_O.tile([128, 65], F32, name="tpO", tag="tps_O")
                    nc.tensor.matmul(
                        pO[:], lhsT=me[:], rhs=v1[:, j, h, :],
                        start=True, stop=True,
                    )
                    rec = np_.tile([128, 1], F32, name="trec", tag="trec")
                    nc.vector.reciprocal(rec[:], pO[:, 64:65])
                    nc.vector.tensor_tensor(
                        rec[:], rec[:], mgs[:, j, 12 + h : 13 + h], op=OP.mult
                    )
                    nc.vector.tensor_scalar_mul(
                        o_t[:, j, 64 * h : 64 * h + 64], pO[:, 0:64], rec[:]
                    )
            # ---- transpose o -> o_f, then Wo and residual add
            o_f = tn_f
            for kt in range(KT):
                pt = ps_tr.tile([128, 256], F32R, name="tpto", tag="tps_tr")
                for j in range(2):
                    nc.tensor.transpose(
                        pt[:, j * 128 : (j + 1) * 128],
                        o_t[:, j, kt * 128 : (kt + 1) * 128],
                        ident[:],
                    )
                nc.scalar.copy(o_f[:, kt, :], pt[:].bitcast(F32))
            for j in range(2):
                for nh in range(2):
                    px = ps_pj.tile([128, 384], F32, name="tpx", tag="tps_pj")
                    for kt in range(KT):
                        nc.tensor.matmul(
                            px[:],
                            lhsT=o_f[:, kt, j * 128 : (j + 1) * 128],
                            rhs=wo[:, kt, nh * 384 : (nh + 1) * 384],
                            start=(kt == 0),
                            stop=(kt == KT - 1),
                        )
                    xs = x_sb[:, ds(off + j, 1), nh * 384 : (nh + 1) * 384]
                    xs = xs.squeeze(1)
                    nc.vector.tensor_tensor(xs, xs, px[:], op=OP.add)

        for _gv in range(8):
            grp_body(_gv)


def _ff_layer(nc, tc, L, x_sb, ident, Win3, Wout3, pfx):
    with ExitStack() as ctx:
        wop = ctx.enter_context(tc.tile_pool(name=f"{pfx}wop{L}", bufs=1))
        wout = wop.tile([128, 16, 768], F32R, name=f"{pfx}wout_t{L}")
        nc.sync.dma_start(
            wout[:], Wout3[L].rearrange("(kt p) m -> p kt m", p=128)
        )
        winp = ctx.enter_context(tc.tile_pool(name=f"{pfx}winp{L}", bufs=2))
        sp = ctx.enter_context(tc.tile_pool(name=f"{pfx}fsp{L}", bufs=1))
        up = ctx.enter_context(tc.tile_pool(name=f"{pfx}fup{L}", bufs=1))
        np_ = ctx.enter_context(tc.tile_pool(name=f"{pfx}fnp{L}", bufs=2))
        ps_tr = ctx.enter_context(
            tc.tile_pool(name=f"{pfx}fps_tr{L}", bufs=2, space="PSUM")
        )
        ps_h = ctx.enter_context(
            tc.tile_pool(name=f"{pfx}fps_h{L}", bufs=4, space="PSUM")
        )
        ps_xd = ctx.enter_context(
            tc.tile_pool(name=f"{pfx}fps_xd{L}", bufs=2, space="PSUM")
        )

        def chunk_body(cv):
            coff = cv * 4
            ss = np_.tile([128, 4], F32, name="ss2", tag="ss2")
            sq = sp.tile([128, 768], F32, name="fsq", tag="fsq")
            for j in range(4):
                nc.scalar.activation(
                    sq[:], x_sb[:, ds(coff + j, 1), :].squeeze(1), AF.Square,
                    accum_out=ss[:, j : j + 1],
                )
            inv = np_.tile([128, 4], F32, name="inv2", tag="inv2")
            _emit_rsqrt(nc, np_, inv[:], ss[:], 1.0 / 768.0, 1e-6, 1e-30)
            tn2 = sp.tile([128, 4, 768], F32R, name="tn2", tag="tn2")
            for j in range(4):
                nc.vector.tensor_scalar_mul(
                    tn2[:, j, :], x_sb[:, ds(coff + j, 1), :].squeeze(1),
                    inv[:, j : j + 1],
                )
            tn2f = sp.tile([128, KT, 512], F32R, name="tn2f", tag="tn2f")
            for kt in range(KT):
                pt = ps_tr.tile([128, 512], F32R, name="fpt", tag="fps_tr")
                for j in range(4):
                    nc.tensor.transpose(
                        pt[:, j * 128 : (j + 1) * 128],
                        tn2[:, j, kt * 128 : (kt + 1) * 128],
                        ident[:],
                    )
                nc.scalar.copy(tn2f[:, kt, :], pt[:].bitcast(F32))
            # ---- h = tn2 @ Win; u = a * gelu(g)
            u = up.tile([128, 16, 512], F32R, name="u", tag="u")
            gl = sp.tile([128, 512], F32, name="gl", tag="gl")
            for m in range(16):
                wa = winp.tile([128, KT, 128], F32R, name="wa", tag="wa")
                wg = winp.tile([128, KT, 128], F32R, name="wg", tag="wg")
                nc.sync.dma_start(
                    wa[:],
                    Win3[L, :, m * 128 : (m + 1) * 128].rearrange(
                        "(kt p) m -> p kt m", p=128
                    ),
                )
                nc.sync.dma_start(
                    wg[:],
                    Win3[L, :, 2048 + m * 128 : 2048 + (m + 1) * 128].rearrange(
                        "(kt p) m -> p kt m", p=128
                    ),
                )
                pa = ps_h.tile([128, 512], F32, name="pa", tag="fps_h")
                pg = ps_h.tile([128, 512], F32, name="pg", tag="fps_h")
                for kt in range(KT):
                    nc.tensor.matmul(
                        pa[:], lhsT=wa[:, kt, :], rhs=tn2f[:, kt, :],
                        start=(kt == 0), stop=(kt == KT - 1),
                    )
                for kt in range(KT):
                    nc.tensor.matmul(
                        pg[:], lhsT=wg[:, kt, :], rhs=tn2f[:, kt, :],
                        start=(kt == 0), stop=(kt == KT - 1),
                    )
                nc.scalar.activation(gl[:], pg[:], AF.Gelu)
                nc.vector.tensor_tensor(u[:, m, :], pa[:], gl[:], op=OP.mult)
            # ---- x += u @ Wout
            for j in range(4):
                for nh in range(2):
                    px = ps_xd.tile([128, 384], F32, name="fpx", tag="fps_xd")
                    for ktf in range(16):
                        nc.tensor.matmul(
                            px[:],
                            lhsT=u[:, ktf, j * 128 : (j + 1) * 128],
                            rhs=wout[:, ktf, nh * 384 : (nh + 1) * 384],
                            start=(ktf == 0),
                            stop=(ktf == 15),
                        )
                    xs = x_sb[:, ds(coff + j, 1), nh * 384 : (nh + 1) * 384]
                    xs = xs.squeeze(1)
                    nc.vector.tensor_tensor(xs, xs, px[:], op=OP.add)

        for _cv in range(4):
            chunk_body(_cv)


def _final_norm(nc, tc, x_sb, x_out):
    xo_view = x_out[:].rearrange("(t p) d -> p t d", p=128)
    with ExitStack() as ctx:
        sp = ctx.enter_context(tc.tile_pool(name="fino", bufs=2))
        np_ = ctx.enter_context(tc.tile_pool(name="finn", bufs=2))
        for cv in range(4):
            coff = cv * 4
            sq = sp.tile([128, 768], F32, name="gsq", tag="gsq")
            ss = np_.tile([128, 4], F32, name="gss", tag="gss")
            for j in range(4):
                nc.scalar.activation(
                    sq[:], x_sb[:, ds(coff + j, 1), :].squeeze(1), AF.Square,
                    accum_out=ss[:, j : j + 1],
                )
            inv = np_.tile([128, 4], F32, name="ginv", tag="ginv")
            _emit_rsqrt(nc, np_, inv[:], ss[:], 1.0 / 768.0, 1e-6, 1e-30)
            ot = sp.tile([128, 4, 768], F32, name="got", tag="got")
            for j in range(4):
                nc.vector.tensor_scalar_mul(
                    ot[:, j, :], x_sb[:, ds(coff + j, 1), :].squeeze(1),
                    inv[:, j : j + 1],
                )
            nc.sync.dma_start(xo_view[:, ds(coff, 4), :], ot[:])
